# revision 1
# baseline (speedup 1.0000x reference)
"""Trainium2 Bass kernel for nn_AngularMultiCenterEmotionBall.

Data-parallel over batch B=16384 across 8 NeuronCores (2048 rows/core).
Each core computes, for its batch shard:
  - LayerNorm(z): stats via bn_stats on row-major z; the (z - mu) @ W
    product is computed from a host-pretransposed zT feed with the mean
    folded in as a rank-1 correction row of the matmul, and the 1/std
    scale folded into the PSUM->SBUF copies. gamma/beta are folded into
    the projection weights on the host.
  - one fused bf16 matmul z0 @ [W_sh | W_sp | W_sh @ c_norm.T] (1024x1052)
  - row norms of z_sh, per-sample center sims -> softmax q, relu(dist_w-r_w)
  - segment statistics (sum_q, sum q*log q, counts) via one-hot mask matmuls
  - partial cross-correlation  z_sh.T @ z_sp  [768, 256]
  - partial column sums/squares of z_sh, z_sp
The host sums the 8 partial outputs and finishes the tiny scalar math
(plus the centers-only overlap/diversity losses).
"""

import os
import sys

import numpy as np

sys.path.insert(0, "/opt/trn_rl_repo")

# problem constants (hardcoded per harness contract)
B, ZD, C, K = 16384, 1024, 7, 4
DSH, DSP = 768, 256
TAU = 0.15
NCORES = 8
BL = B // NCORES          # 2048 rows per core
P = 128
NT = BL // P              # 16 row-tiles per core
CK = C * K                # 28
NW = DSH + DSP + CK       # 1052 fused output columns
KC = ZD // P              # 8 contraction chunks

_GRAPH_CACHE = {}


def _split_multiwaits(nc):
    """Walrus codegen in this container accepts at most one semaphore wait
    per engine instruction. TileContext attaches several. Peel the extra
    waits off into standalone single-wait EventSemaphore instructions
    (what raw-bass wait_ge emits) placed just before the instruction —
    the engine is in-order, so wait(A); wait(B); op == op waiting {A,B}.
    Applied as a JSON rewrite at serialization time."""
    import json

    orig = nc.to_json_bytes

    def patched():
        d = json.loads(orig())
        ctr = [0]
        for f in d["functions"]:
            for b in f["blocks"]:
                insts = b.get("instructions")
                if not insts:
                    continue
                out = []
                for i in insts:
                    si = i.get("sync_info") or {}
                    waits = si.get("on_wait") or []
                    if len(waits) > 1:
                        for w in waits[:-1]:
                            ctr[0] += 1
                            out.append(
                                {
                                    "engine": i["engine"],
                                    "ins": [],
                                    "name": f"splitwait_{ctr[0]}",
                                    "opcode": "EventSemaphore",
                                    "outs": [],
                                    "sync_info": {
                                        "on_update": [],
                                        "on_wait": [w],
                                    },
                                }
                            )
                        si["on_wait"] = [waits[-1]]
                    out.append(i)
                b["instructions"] = out
        return json.dumps(d).encode()

    nc.to_json_bytes = patched
    return nc


def _build_graph(with_bias: bool):
    import concourse.bass as bass
    import concourse.tile as tile
    from concourse import mybir
    from concourse.masks import make_identity

    f32 = mybir.dt.float32
    b16 = mybir.dt.bfloat16
    AF = mybir.ActivationFunctionType
    ALU = mybir.AluOpType

    nc = bass.Bass()
    z_ext = nc.declare_dram_parameter("z", [BL, ZD], b16, isOutput=False)
    zt_ext = nc.declare_dram_parameter("zt", [NT, ZD, P], b16, isOutput=False)
    w_ext = nc.declare_dram_parameter("w", [9 * P, NW], b16, isOutput=False)
    mk_ext = nc.declare_dram_parameter("mk", [BL, 8], f32, isOutput=False)
    rl_ext = nc.declare_dram_parameter("rl", [BL, K], f32, isOutput=False)
    if with_bias:
        br_ext = nc.declare_dram_parameter("br", [1, NW], f32, isOutput=False)
    o_corr = nc.declare_dram_parameter("o_corr", [DSP, DSH], f32, isOutput=True)
    o_a0 = nc.declare_dram_parameter("o_a0", [1, 512], f32, isOutput=True)
    o_a1 = nc.declare_dram_parameter("o_a1", [1, 512], f32, isOutput=True)
    o_misc = nc.declare_dram_parameter("o_misc", [8, 272], f32, isOutput=True)
    o_intra = nc.declare_dram_parameter("o_intra", [P, NT], f32, isOutput=True)

    with tile.TileContext(nc) as tc:
        with (
            tc.tile_pool(name="singles", bufs=1) as singles,
            tc.tile_pool(name="work", bufs=2) as work,
            tc.tile_pool(name="zin", bufs=4) as zin,
            tc.tile_pool(name="stats", bufs=6) as stats,
            tc.tile_pool(name="outst", bufs=2) as outst,
            tc.tile_pool(name="pmain", bufs=1, space="PSUM") as pmain,
            tc.tile_pool(name="ptr", bufs=2, space="PSUM") as ptr_pool,
            tc.tile_pool(name="pacc", bufs=1, space="PSUM") as pacc,
        ):
            # ---- persistent SBUF state ----
            W_sb = singles.tile([P, 9, NW], b16)
            for kc in range(9):
                nc.scalar.dma_start(
                    out=W_sb[:, kc, :],
                    in_=w_ext[kc * P : (kc + 1) * P, :],
                )
            zT_all = singles.tile([P, KC, BL], b16)
            mask_all = singles.tile([P, NT, 8], f32)
            nc.gpsimd.dma_start(
                out=mask_all, in_=mk_ext[:].rearrange("(t p) c -> p t c", p=P)
            )
            rlab_all = singles.tile([P, NT, K], f32)
            nc.gpsimd.dma_start(
                out=rlab_all, in_=rl_ext[:].rearrange("(t p) k -> p t k", p=P)
            )
            if with_bias:
                br_sb = singles.tile([1, NW], f32)
                nc.sync.dma_start(out=br_sb, in_=br_ext[:])

            ident = singles.tile([P, P], f32)
            make_identity(nc, ident)
            ident_bf = singles.tile([P, P], b16)
            nc.scalar.copy(out=ident_bf, in_=ident)
            eps_t = singles.tile([P, 1], f32)
            nc.gpsimd.memset(eps_t, 1e-5)
            zero_t = singles.tile([P, 1], f32)
            nc.gpsimd.memset(zero_t, 0.0)
            eps8_t = singles.tile([P, 1], f32)
            nc.gpsimd.memset(eps8_t, 1e-8)
            one_t = singles.tile([P, 1], f32)
            nc.gpsimd.memset(one_t, 1.0)
            ones_col = singles.tile([P, 1], b16)
            nc.scalar.copy(out=ones_col, in_=one_t)
            mask_bf = singles.tile([P, NT, 8], b16)
            nc.scalar.copy(out=mask_bf, in_=mask_all)

            z_sh_all = singles.tile([P, NT, DSH], b16)
            z_sp_all = singles.tile([P, NT, DSP], b16)
            sraw_all = singles.tile([P, NT, CK], f32)
            n2_all = singles.tile([P, NT], f32)
            negmu_sb = singles.tile([1, BL], b16)

            # persistent PSUM accumulators. NOTE: regions that accumulate
            # concurrently (interleaved start..stop groups) must be in
            # distinct banks — a start=True clears has_written for the
            # whole bank, so a second group sharing the bank would turn
            # the other group's next accumulate into an overwrite.
            acc0 = pacc.tile([1, 512], f32)    # ssq_sh[0:512]
            acc1 = pacc.tile([1, 512], f32)    # ssq_sh[512:768]
            acc2 = pacc.tile([8, 512], f32)    # [0,0:256]=sum_sp ; [0:8,256:262]=seg (tail)

            def copy_scaled(dst, src_psum, rstd, col0, ncol, engine):
                """PSUM->SBUF move with the folded 1/std LayerNorm scale
                (plus the folded bias row when present)."""
                if engine == "act":
                    nc.scalar.activation(
                        out=dst, in_=src_psum, func=AF.Copy, scale=rstd
                    )
                else:
                    nc.vector.tensor_scalar_mul(dst, src_psum, rstd)
                if with_bias:
                    nc.vector.tensor_tensor(
                        out=dst,
                        in0=dst,
                        in1=br_sb[0:1, col0 : col0 + ncol].partition_broadcast(P),
                        op=ALU.add,
                    )

            # ---- main loop, software-pipelined by one tile ----
            # stats(t+1) and ssq-accumulation(t-1) are emitted around
            # tile t's matmul group so the in-order PE stream never
            # waits on the bn-stats chain or the ACT squares.
            rstds = [None] * NT
            sqhs = [None] * NT

            def emit_ztload(t):
                nc.sync.dma_start(
                    out=zT_all[:, :, t * P : (t + 1) * P],
                    in_=zt_ext[t].rearrange("(o p) b -> p o b", p=P),
                )

            def emit_stats(t):
                ts_ = slice(t * P, (t + 1) * P)
                zt = zin.tile([P, ZD], b16, name="zt")
                nc.gpsimd.dma_start(out=zt, in_=z_ext[ts_, :])
                st = stats.tile([P, 2, 6], f32, name="st")
                nc.vector.bn_stats(out=st[:, 0, :], in_=zt[:, 0:512])
                nc.vector.bn_stats(out=st[:, 1, :], in_=zt[:, 512:1024])
                mv = stats.tile([P, 2], f32, name="mv")
                nc.vector.bn_aggr(out=mv, in_=st)
                stdt = stats.tile([P, 1], f32, name="stdt")
                nc.scalar.activation(
                    out=stdt, in_=mv[:, 1:2], func=AF.Sqrt, bias=eps_t, scale=1.0
                )
                rstd = stats.tile([P, 1], f32, name="rstd")
                nc.vector.reciprocal(out=rstd, in_=stdt)
                rstds[t] = rstd
                mub = stats.tile([P, 1], b16, name="mub")
                nc.scalar.activation(
                    out=mub, in_=mv[:, 0:1], func=AF.Copy, scale=-1.0
                )
                ptr_mu = ptr_pool.tile([P, 512], b16, tag="tr", name="ptr_mu")
                nc.tensor.transpose(ptr_mu[0:1, 0:P], mub, ident_bf)
                nc.scalar.copy(out=negmu_sb[0:1, ts_], in_=ptr_mu[0:1, 0:P])

            def emit_mm(t):
                ts_ = slice(t * P, (t + 1) * P)
                pA = pmain.tile([P, 512], f32, tag="mA", name="pA")
                pB = pmain.tile([P, 512], f32, tag="mB", name="pB")
                pC = pmain.tile([P, CK], f32, tag="mC", name="pC")
                for kc in range(KC):
                    lhsT = zT_all[:, kc, ts_]
                    first = kc == 0
                    nc.tensor.matmul(
                        pA, lhsT, W_sb[:, kc, 0:512], start=first, stop=False
                    )
                    nc.tensor.matmul(
                        pB, lhsT, W_sb[:, kc, 512:1024], start=first, stop=False
                    )
                    nc.tensor.matmul(
                        pC, lhsT, W_sb[:, kc, 1024:NW], start=first, stop=False
                    )
                # rank-1 LayerNorm mean correction: += (-mu) x colsum(W)
                cmu = negmu_sb[0:1, ts_]
                nc.tensor.matmul(
                    pA, cmu, W_sb[0:1, 8, 0:512], start=False, stop=True
                )
                nc.tensor.matmul(
                    pB, cmu, W_sb[0:1, 8, 512:1024], start=False, stop=True
                )
                nc.tensor.matmul(
                    pC, cmu, W_sb[0:1, 8, 1024:NW], start=False, stop=True
                )
                return pA, pB, pC

            def emit_copies(t, pA, pB, pC):
                rstd = rstds[t]
                copy_scaled(z_sh_all[:, t, 0:512], pA, rstd, 0, 512, "act")
                copy_scaled(z_sh_all[:, t, 512:768], pB[:, 0:256], rstd, 512, 256, "dve")
                copy_scaled(z_sp_all[:, t, :], pB[:, 256:512], rstd, 768, 256, "dve")
                copy_scaled(sraw_all[:, t, :], pC, rstd, 1024, CK, "act")
                sqh = work.tile([P, DSH], b16, tag="sqh", name="sqh")
                nc.scalar.activation(
                    out=sqh, in_=z_sh_all[:, t, :], func=AF.Square,
                    bias=zero_t,
                    accum_out=n2_all[:, t : t + 1],
                )
                sqhs[t] = sqh

            def emit_ssq(t):
                fl = t == 0
                ll = t == NT - 1
                sqh = sqhs[t]
                nc.tensor.matmul(
                    acc0[0:1, :], ones_col, sqh[:, 0:512],
                    start=fl, stop=ll, skip_group_check=True,
                )
                nc.tensor.matmul(
                    acc1[0:1, 0:256], ones_col, sqh[:, 512:768],
                    start=fl, stop=ll, skip_group_check=True,
                )
                nc.tensor.matmul(
                    acc2[0:1, 0:256], ones_col, z_sp_all[:, t, :],
                    start=fl, stop=ll, skip_group_check=True,
                )

            for t0 in range(4):
                emit_ztload(t0)
            emit_stats(0)
            emit_stats(1)
            for t in range(NT):
                if t >= 1:
                    emit_ssq(t - 1)
                mm = emit_mm(t)
                if t + 4 < NT:
                    emit_ztload(t + 4)
                if t + 2 < NT:
                    emit_stats(t + 2)
                emit_copies(t, *mm)
            emit_ssq(NT - 1)

            # ---- batched softmax / loss tail over [128, 16, *] ----
            nrm = stats.tile([P, NT], f32, tag="nrm")
            nc.scalar.activation(out=nrm, in_=n2_all, func=AF.Sqrt, bias=zero_t)
            nc.vector.tensor_scalar_max(nrm, nrm, 1e-12)
            rn = stats.tile([P, NT], f32, tag="rn")
            nc.vector.reciprocal(out=rn, in_=nrm)

            sim_all = singles.tile([P, NT, CK], f32)
            nc.vector.tensor_tensor(
                out=sim_all, in0=sraw_all,
                in1=rn[:, :, None].to_broadcast([P, NT, CK]), op=ALU.mult,
            )
            # gather label class: simK[p,t,k] = sum_c mask[p,t,c] * sim[p,t,c*4+k]
            t47 = singles.tile([P, NT, K, C], f32)
            nc.vector.tensor_tensor(
                out=t47,
                in0=sim_all.rearrange("p t (c k) -> p t k c", k=K),
                in1=mask_all[:, :, None, 0:C].to_broadcast([P, NT, K, C]),
                op=ALU.mult,
            )
            simK = singles.tile([P, NT, K], f32)
            nc.vector.reduce_sum(out=simK, in_=t47, axis=mybir.AxisListType.X)

            mx = stats.tile([P, NT], f32, tag="mx")
            nc.vector.reduce_max(out=mx, in_=simK, axis=mybir.AxisListType.X)
            dsub = singles.tile([P, NT, K], f32)
            nc.vector.tensor_tensor(
                out=dsub, in0=simK,
                in1=mx[:, :, None].to_broadcast([P, NT, K]), op=ALU.subtract,
            )
            e_all = singles.tile([P, NT, K], f32)
            nc.scalar.activation(
                out=e_all, in_=dsub, func=AF.Exp, scale=1.0 / TAU, bias=zero_t
            )
            se = stats.tile([P, NT], f32, tag="se")
            nc.vector.reduce_sum(out=se, in_=e_all, axis=mybir.AxisListType.X)
            rse = stats.tile([P, NT], f32, tag="rse")
            nc.vector.reciprocal(out=rse, in_=se)
            q_all = singles.tile([P, NT, K], f32)
            nc.vector.tensor_tensor(
                out=q_all, in0=e_all,
                in1=rse[:, :, None].to_broadcast([P, NT, K]), op=ALU.mult,
            )
            lg = singles.tile([P, NT, K], f32)
            nc.scalar.activation(out=lg, in_=q_all, func=AF.Ln, bias=eps8_t)
            ql = singles.tile([P, NT, K], f32)
            nc.vector.tensor_tensor(out=ql, in0=q_all, in1=lg, op=ALU.mult)
            qlsum = stats.tile([P, NT], f32, tag="qlsum")
            nc.vector.reduce_sum(out=qlsum, in_=ql, axis=mybir.AxisListType.X)

            qs = singles.tile([P, NT, K], f32, tag="qs")
            nc.vector.tensor_tensor(out=qs, in0=q_all, in1=simK, op=ALU.mult)
            ds = stats.tile([P, NT], f32, tag="ds")
            nc.vector.reduce_sum(out=ds, in_=qs, axis=mybir.AxisListType.X)
            qr = singles.tile([P, NT, K], f32, tag="qr")
            nc.vector.tensor_tensor(out=qr, in0=q_all, in1=rlab_all, op=ALU.mult)
            rw = stats.tile([P, NT], f32, tag="rw")
            nc.vector.reduce_sum(out=rw, in_=qr, axis=mybir.AxisListType.X)
            s_all = stats.tile([P, NT], f32, tag="s_all")
            nc.vector.tensor_tensor(out=s_all, in0=ds, in1=rw, op=ALU.add)
            strip = singles.tile([P, NT], f32)
            # relu(dist_w - r_w) = Relu(1 - (ds + rw))
            nc.scalar.activation(
                out=strip, in_=s_all, func=AF.Relu, scale=-1.0, bias=one_t
            )
            nc.sync.dma_start(out=o_intra[:], in_=strip)

            # ---- tail: ssq_sp via sequential accumulation (own tr bank) ----
            psp_full = ptr_pool.tile([P, 512], f32, tag="tr")
            for t in range(NT):
                sqp = work.tile([P, DSP], b16, tag="sqp")
                nc.scalar.activation(
                    out=sqp, in_=z_sp_all[:, t, :], func=AF.Square, bias=zero_t
                )
                nc.tensor.matmul(
                    psp_full[0:1, 0:256], ones_col, sqp,
                    start=(t == 0), stop=(t == NT - 1), skip_group_check=True,
                )
            a1s = outst.tile([1, 512], f32, tag="a1s")
            nc.scalar.copy(out=a1s[0:1, 0:256], in_=acc1[0:1, 0:256])
            nc.scalar.copy(out=a1s[0:1, 256:512], in_=psp_full[0:1, 0:256])
            nc.sync.dma_start(out=o_a1[:], in_=a1s)

            # ---- cross-correlation tail (transposed layout):
            # corrT[j, i] = sum_b z_sp[b, j] * z_sh[b, i]; z_sp chunks are
            # the stationary operand so the moving stream is 512 wide.
            for jc in range(DSP // P):
                pj0 = ptr_pool.tile([P, 512], f32, tag="tr", name="pj0")
                pj1_full = ptr_pool.tile([P, 512], f32, tag="tr", name="pj1")
                pj1 = pj1_full[:, 0:256]
                for t in range(NT):
                    lhsT = z_sp_all[:, t, jc * P : (jc + 1) * P]
                    nc.tensor.matmul(
                        pj0, lhsT, z_sh_all[:, t, 0:512],
                        start=(t == 0), stop=(t == NT - 1),
                        skip_group_check=True,
                    )
                    nc.tensor.matmul(
                        pj1, lhsT, z_sh_all[:, t, 512:768],
                        start=(t == 0), stop=(t == NT - 1),
                        skip_group_check=True,
                    )
                ct = outst.tile([P, DSH], f32, tag="ct")
                nc.scalar.copy(out=ct[:, 0:512], in_=pj0)
                nc.scalar.copy(out=ct[:, 512:768], in_=pj1)
                nc.sync.dma_start(out=o_corr[jc * P : (jc + 1) * P, :], in_=ct)

            # segment-sum matmuls: acc2[c, 256+j] += sum_b mask[b,c]*R[b,j]
            R_all = singles.tile([P, NT, 6], b16)
            nc.scalar.copy(out=R_all[:, :, 0:4], in_=q_all)
            nc.scalar.copy(out=R_all[:, :, 4:5], in_=qlsum[:, :, None])
            nc.scalar.copy(
                out=R_all[:, :, 5:6],
                in_=one_t[:, None, 0:1].to_broadcast([P, NT, 1]),
            )
            for t in range(NT):
                nc.tensor.matmul(
                    acc2[:, 256:262],
                    mask_bf[:, t, :],
                    R_all[:, t, :],
                    start=(t == 0), stop=(t == NT - 1),
                    skip_group_check=True,
                )

            # ---- epilogue: accumulators -> SBUF -> DRAM ----
            a0s = outst.tile([1, 512], f32, tag="a0s")
            nc.scalar.copy(out=a0s, in_=acc0)
            nc.sync.dma_start(out=o_a0[:], in_=a0s)
            ms = outst.tile([8, 272], f32, tag="ms")
            nc.gpsimd.memset(ms, 0.0)
            nc.scalar.copy(out=ms[0:1, 0:256], in_=acc2[0:1, 0:256])
            nc.scalar.copy(out=ms[:, 256:262], in_=acc2[:, 256:262])
            nc.sync.dma_start(out=o_misc[:], in_=ms)

    return _split_multiwaits(nc)


def _host_prep(inputs):
    import ml_dtypes

    bf16 = ml_dtypes.bfloat16
    z = np.asarray(inputs["z"], dtype=np.float32)
    labels = np.asarray(inputs["labels"]).astype(np.int64)
    gamma = np.asarray(inputs["ln_gamma"], dtype=np.float32)
    beta = np.asarray(inputs["ln_beta"], dtype=np.float32)
    W_sh = np.asarray(inputs["W_sh"], dtype=np.float32)
    b_sh = np.asarray(inputs["b_sh"], dtype=np.float32)
    W_sp = np.asarray(inputs["W_sp"], dtype=np.float32)
    b_sp = np.asarray(inputs["b_sp"], dtype=np.float32)
    centers = np.asarray(inputs["centers"], dtype=np.float32)
    radii = np.asarray(inputs["ema_radii"], dtype=np.float32)

    cf = centers.reshape(CK, DSH)
    cn = cf / np.maximum(
        np.linalg.norm(cf, axis=1, keepdims=True), 1e-12
    ).astype(np.float32)
    W_all = np.concatenate([W_sh, W_sp, W_sh @ cn.T], axis=1)  # [ZD, NW]
    W_eff = (gamma[:, None] * W_all).astype(np.float32)
    # row 1024: column sums for the rank-1 (-mu) LayerNorm correction;
    # rows 1025..1151: zero pad to 9*128 for the [128, 9, NW] SBUF layout.
    W_ext = np.zeros((9 * P, NW), np.float32)
    W_ext[:ZD] = W_eff
    W_ext[ZD] = W_eff.sum(0)
    W_bf = np.ascontiguousarray(W_ext.astype(bf16))

    be_sh = beta @ W_sh + b_sh
    be_sp = beta @ W_sp + b_sp
    b_eff = np.concatenate([be_sh, be_sp, be_sh @ cn.T]).astype(np.float32)
    with_bias = bool(np.any(b_eff != 0.0))

    onehot = (labels[:, None] == np.arange(8)[None, :]).astype(np.float32)
    rlab = radii.reshape(CK // K, K)[labels].astype(np.float32)  # [B, K]
    z_bf = z.astype(bf16)

    in_maps = []
    for i in range(NCORES):
        sl = slice(i * BL, (i + 1) * BL)
        m = {
            "z": np.ascontiguousarray(z_bf[sl]),
            "zt": np.ascontiguousarray(
                z_bf[sl].T.reshape(ZD, NT, P).transpose(1, 0, 2)
            ),
            "w": W_bf,
            "mk": np.ascontiguousarray(onehot[sl]),
            "rl": np.ascontiguousarray(rlab[sl]),
        }
        if with_bias:
            m["br"] = np.ascontiguousarray(b_eff[None, :])
        in_maps.append(m)
    return in_maps, with_bias, cn


def _host_finish(results, cn):
    f64 = np.float64
    corr_raw = np.zeros((DSH, DSP), f64)
    a0 = np.zeros(512, f64)
    a1 = np.zeros(512, f64)
    sum_sp = np.zeros(DSP, f64)
    seg = np.zeros((8, 6), f64)
    intra_sum = 0.0
    for r in results:
        corr_raw += r["o_corr"].T.astype(f64)
        a0 += r["o_a0"][0].astype(f64)
        a1 += r["o_a1"][0].astype(f64)
        sum_sp += r["o_misc"][0, 0:256].astype(f64)
        seg += r["o_misc"][:, 256:262].astype(f64)
        intra_sum += float(r["o_intra"].astype(f64).sum())

    ssq_sh = np.concatenate([a0, a1[0:256]])
    ssq_sp = a1[256:512]
    sum_q = seg[0:C, 0:K]
    qlsum_c = seg[0:C, 4]
    counts = seg[0:C, 5]

    n_sh = np.maximum(np.sqrt(ssq_sh), 1e-12)
    n_sp = np.maximum(np.sqrt(ssq_sp), 1e-12)
    corr = corr_raw / np.outer(n_sh, n_sp)
    L_ortho = (corr**2).mean()

    v = ssq_sp / B - (sum_sp / B) ** 2
    L_var = np.maximum(0.05 - v, 0.0).mean()

    L_intra = intra_sum / B

    p = sum_q / (sum_q.sum(-1, keepdims=True) + 1e-8)
    H_marg = -(p * np.log(p + 1e-8)).sum(-1)
    H_cond = (-qlsum_c) / np.maximum(counts, 1.0)
    valid = counts > 0
    L_bal_k = np.log(f64(K)) - H_marg + H_cond
    L_balance = np.where(valid, L_bal_k, 0.0).sum() / max(int(valid.sum()), 1)

    sim_mat = (cn @ cn.T).astype(f64)
    blkmask = 1.0 - np.kron(np.eye(C), np.ones((K, K)))
    L_overlap = (np.maximum(sim_mat - 0.3, 0.0) * blkmask).sum() / (
        blkmask.sum() + 1e-6
    )
    cnr = cn.reshape(C, K, DSH).astype(f64)
    sims_in = np.einsum("ckd,cld->ckl", cnr, cnr)
    triu = np.triu(np.ones((K, K)), 1)
    L_div = (np.maximum(sims_in - 0.8, 0.0) * triu).sum() / max(
        C * K * (K - 1) // 2, 1
    )

    L_ball = L_intra + 0.3 * L_overlap + 0.2 * L_div + 0.15 * L_balance
    loss = L_ball + 0.02 * L_ortho + 0.005 * L_var
    return np.float32(loss)


def _run_hw(nc, in_maps, trace=False, tmpdir=None):
    from concourse.bass_utils import run_bass_kernel_spmd

    res = run_bass_kernel_spmd(
        nc, in_maps, core_ids=list(range(NCORES)), trace=trace, tmpdir=tmpdir
    )
    return res


def _run_sim(nc, in_maps):
    from concourse.bass_interp import CoreSim

    outs = []
    for i, im in enumerate(in_maps):
        sim = CoreSim(nc, publish_trace=False)
        sim.assign_tensors(im)
        sim.simulate()
        outs.append(
            {k: np.array(sim.tensor(k)) for k in
             ("o_corr", "o_a0", "o_a1", "o_misc", "o_intra")}
        )
    return outs


def kernel(**inputs) -> np.ndarray:
    in_maps, with_bias, cn = _host_prep(inputs)
    if with_bias not in _GRAPH_CACHE:
        _GRAPH_CACHE[with_bias] = _build_graph(with_bias)
    nc = _GRAPH_CACHE[with_bias]
    if os.environ.get("KERNEL_BASS_SIM"):
        results = _run_sim(nc, in_maps)
    else:
        results = _run_hw(nc, in_maps).results
    return _host_finish(results, cn)



# revision 24
# speedup vs baseline: 1.5098x; 1.5098x over previous
"""Trainium2 Bass kernel for nn_AngularMultiCenterEmotionBall.

Data-parallel over batch B=16384 across 8 NeuronCores (2048 rows/core).

The projection GEMM z0 @ [W_sh | W_sp | W_sh @ c_norm.T] runs in fp8
(e4m3) with MatmulPerfMode.DoubleRow: each matmul instruction contracts
2x128 rows, doubling PE throughput vs bf16.  The host centers z
(mean-subtract, a shift the projection is equivariant to) and the
per-row 1/std LayerNorm scale is computed on-device via bn_stats and
folded into the PSUM->SBUF copies.  gamma/beta are folded into the
projection weights on the host (beta == 0 here).

Per-core device work:
  - bn_stats over centered-z rows -> rstd (the only LN stat left)
  - fp8 DoubleRow GEMM  (zc/std) @ [W_sh | W_sp | W_sh cnT] * scales
  - z_sh/z_sp stored fp8 at 16x natural scale; squares at 9x/8x via
    one ACT Square (with accum_out row-norms) and one Pool STT
  - column sums-of-squares / sums via ones-stationary DoubleRow
    matmuls packed into ONE PSUM bank at out partitions {0,32,64}
    (bank pre-cleared by a zeros-stationary matmul)
  - per-sample softmax q over the label's 4 centers, relu(dist-r)
  - segment stats (sum_q, sum q log q, counts) via one-hot matmuls
  - cross-correlation z_sp.T @ [z_sh | z_sp] in fp8 DoubleRow (the
    z_sp Gram diagonal supplies ssq_sp for the variance-floor loss)
The host sums the 8 partial outputs and finishes the scalar math
(plus the centers-only overlap/diversity losses).
"""

import os
import sys

import numpy as np

sys.path.insert(0, "/opt/trn_rl_repo")

# problem constants (hardcoded per harness contract)
B, ZD, C, K = 16384, 1024, 7, 4
DSH, DSP = 768, 256
TAU = 0.15
NCORES = 8
BL = B // NCORES          # 2048 rows per core
P = 128
NT = BL // P              # 16 row-tiles per core
CK = C * K                # 28
NW = DSH + DSP + CK       # 1052 fused output columns
KC = ZD // P              # 8 contraction chunks
G = KC // 2               # 4 DoubleRow groups (256 contraction each)
NPR = NT // 2             # 8 tile pairs
CHT = 4                   # tiles per softmax chunk
NCH = NT // CHT           # 4 chunks

S_W = 64.0                # fp8 weight scale
S_H = 16.0                # fp8 z_sh/z_sp storage scale
SQ_S = 0.1875             # ACT Square input scale: sqh = 9 * z_sh^2
SQP_S = 1.0 / 32.0        # Pool STT scale:        sqp = 8 * z_sp^2
NRM_S = (S_H * S_H) / (S_H * SQ_S) ** 2   # 256/9: nrm = 16*||z_sh||

_GRAPH_CACHE = {}


def _split_multiwaits(nc):
    """Walrus codegen in this container accepts at most one semaphore wait
    per engine instruction. TileContext attaches several. Peel the extra
    waits off into standalone single-wait EventSemaphore instructions
    (what raw-bass wait_ge emits) placed just before the instruction —
    the engine is in-order, so wait(A); wait(B); op == op waiting {A,B}.
    Applied as a JSON rewrite at serialization time."""
    import json

    orig = nc.to_json_bytes

    def patched():
        d = json.loads(orig())
        ctr = [0]
        for f in d["functions"]:
            for b in f["blocks"]:
                insts = b.get("instructions")
                if not insts:
                    continue
                out = []
                for i in insts:
                    si = i.get("sync_info") or {}
                    waits = si.get("on_wait") or []
                    if len(waits) > 1:
                        for w in waits[:-1]:
                            ctr[0] += 1
                            out.append(
                                {
                                    "engine": i["engine"],
                                    "ins": [],
                                    "name": f"splitwait_{ctr[0]}",
                                    "opcode": "EventSemaphore",
                                    "outs": [],
                                    "sync_info": {
                                        "on_update": [],
                                        "on_wait": [w],
                                    },
                                }
                            )
                        si["on_wait"] = [waits[-1]]
                    out.append(i)
                b["instructions"] = out
        return json.dumps(d).encode()

    nc.to_json_bytes = patched
    return nc


def _build_graph(with_bias: bool):
    import concourse.bass as bass
    import concourse.tile as tile
    from concourse import mybir

    f32 = mybir.dt.float32
    b16 = mybir.dt.bfloat16
    f8 = mybir.dt.float8e4
    AF = mybir.ActivationFunctionType
    ALU = mybir.AluOpType
    DR = mybir.MatmulPerfMode.DoubleRow
    AX = mybir.AxisListType.X

    nc = bass.Bass()
    zt_ext = nc.declare_dram_parameter("zt", [NT, P, KC * P], f8, isOutput=False)
    zr_ext = nc.declare_dram_parameter("zr", [BL, ZD], b16, isOutput=False)
    w_ext = nc.declare_dram_parameter("w", [P, 2 * G * NW], f8, isOutput=False)
    mk_ext = nc.declare_dram_parameter("mk", [BL, 8], f32, isOutput=False)
    rl_ext = nc.declare_dram_parameter("rl", [BL, K], f32, isOutput=False)
    if with_bias:
        br_ext = nc.declare_dram_parameter("br", [1, NW], f32, isOutput=False)
    o_corr = nc.declare_dram_parameter("o_corr", [DSP, DSH], b16, isOutput=True)
    o_gram = nc.declare_dram_parameter("o_gram", [DSP, DSP], b16, isOutput=True)
    o_stat = nc.declare_dram_parameter("o_stat", [1, 1024], f32, isOutput=True)
    o_seg = nc.declare_dram_parameter("o_seg", [8, 6], f32, isOutput=True)
    o_intra = nc.declare_dram_parameter("o_intra", [P, NT], f32, isOutput=True)

    with tile.TileContext(nc) as tc:
        with (
            tc.tile_pool(name="singles", bufs=1) as singles,
            tc.tile_pool(name="zin", bufs=1) as zin,
            tc.tile_pool(name="stats", bufs=6) as stats,
            tc.tile_pool(name="cpool", bufs=2) as cpool,
            tc.tile_pool(name="outst", bufs=2) as outst,
            tc.tile_pool(name="pA", bufs=2, space="PSUM") as pA_pool,
            tc.tile_pool(name="pB", bufs=2, space="PSUM") as pB_pool,
            tc.tile_pool(name="pC", bufs=2, space="PSUM") as pC_pool,
            tc.tile_pool(name="pacc", bufs=1, space="PSUM") as pacc,
        ):
            # ---- persistent SBUF state ----
            W_sb = singles.tile([P, 2 * G, NW], f8)
            zT_all = singles.tile([P, NT, KC * P], f8)
            zshsp = singles.tile([P, NT, 1024], f8)   # [z_sh 768 | z_sp 256]
            sq_all = singles.tile([P, NT, DSH], f8)   # 9 * z_sh^2
            sraw_all = singles.tile([P, NT, CK], f32)  # 16x natural sims
            n2_all = singles.tile([P, NT], f32)
            mask_all = singles.tile([P, NT, 8], f32)
            mask_bf = singles.tile([P, NT, 8], b16)
            rlab_all = singles.tile([P, NT, K], f32)
            R_all = singles.tile([P, NT, 6], b16)
            strip_all = singles.tile([P, NT], f32)

            # ---- input DMAs, ordered for early steady-state ----
            zr_tiles = {
                t: zin.tile([P, 2, 512], b16, name=f"zr{t}", tag=f"zr{t}")
                for t in range(NT)
            }

            def dma_zr(engine, t):
                engine.dma_start(
                    out=zr_tiles[t],
                    in_=zr_ext[t * P : (t + 1) * P, :].rearrange(
                        "b (g f) -> b g f", g=2
                    ),
                )

            # sync queue: z-transposed tiles + row-major even tiles
            for t in range(NT):
                if t % 2 == 0:
                    dma_zr(nc.sync, t)
                nc.sync.dma_start(out=zT_all[:, t, :], in_=zt_ext[t])
            # gpsimd queue: weights first, then row-major odd tiles, masks
            gp_order = []
            for g in range(G):
                gp_order.append(("w", g))
                if g < 2:
                    gp_order.append(("zr", 2 * g + 1))
            for t in range(5, NT, 2):
                gp_order.append(("zr", t))
            gp_order.append(("mk", 0))
            gp_order.append(("rl", 0))
            for kind, idx in gp_order:
                if kind == "w":
                    nc.gpsimd.dma_start(
                        out=W_sb[:, 2 * idx : 2 * idx + 2, :],
                        in_=w_ext[
                            :, 2 * idx * NW : (2 * idx + 2) * NW
                        ].rearrange("p (j c) -> p j c", j=2),
                    )
                elif kind == "zr":
                    dma_zr(nc.gpsimd, idx)
                elif kind == "mk":
                    nc.gpsimd.dma_start(
                        out=mask_all,
                        in_=mk_ext[:].rearrange("(t p) c -> p t c", p=P),
                    )
                else:
                    nc.gpsimd.dma_start(
                        out=rlab_all,
                        in_=rl_ext[:].rearrange("(t p) k -> p t k", p=P),
                    )
            if with_bias:
                br_sb = singles.tile([1, NW], f32)
                nc.vector.dma_start(out=br_sb, in_=br_ext[:])

            # constants
            zero_t = singles.tile([P, 1], f32)
            nc.gpsimd.memset(zero_t, 0.0)
            one_t = singles.tile([P, 1], f32)
            nc.gpsimd.memset(one_t, 1.0)
            eps8_t = singles.tile([P, 1], f32)
            nc.gpsimd.memset(eps8_t, 1e-8)
            seps_t = singles.tile([P, 1], f32)
            nc.gpsimd.memset(seps_t, (S_W / S_H) ** 2 * 1e-5)
            # ones-in-column-0 stationary: PE stationary tiles are 32-wide
            # minimum, and DoubleRow only works at tile position (0, 0), so
            # each stat sum lands on out partitions 0:32 with row 0 live.
            ones32 = singles.tile([P, 2, 32], f8)
            nc.gpsimd.memset(ones32, 0.0)
            nc.scalar.copy(
                out=ones32[:, :, 0:1],
                in_=one_t[:, None, 0:1].to_broadcast([P, 2, 1]),
            )

            nc.scalar.copy(out=mask_bf, in_=mask_all)
            nc.scalar.copy(
                out=R_all[:, :, 5:6],
                in_=one_t[:, None, 0:1].to_broadcast([P, NT, 1]),
            )

            rstds = [None] * NT

            def emit_stats(t, zrt):
                st = stats.tile([P, 2, 6], b16, name="st")
                nc.vector.bn_stats(out=st[:, 0, :], in_=zrt[:, 0, :])
                nc.vector.bn_stats(out=st[:, 1, :], in_=zrt[:, 1, :])
                mv = stats.tile([P, 2], f32, name="mv")
                nc.vector.bn_aggr(out=mv, in_=st)
                stdt = stats.tile([P, 1], f32, name="stdt")
                # rstd_eff = S_H/(S_W*std) = 1/sqrt((S_W/S_H)^2*(var+eps))
                nc.scalar.activation(
                    out=stdt, in_=mv[:, 1:2], func=AF.Sqrt,
                    bias=seps_t, scale=(S_W / S_H) ** 2,
                )
                rstd = stats.tile([P, 1], f32, name="rstd")
                nc.vector.reciprocal(out=rstd, in_=stdt)
                rstds[t] = rstd

            def emit_mm(t):
                pA = pA_pool.tile([P, 512], f32, tag="mA", name="pA")
                pB = pB_pool.tile([P, 512], f32, tag="mB", name="pB")
                # full-bank tile: a sub-bank tile would share its bank with
                # the other buf, and start=True clears has_written bank-wide
                pC = pC_pool.tile([P, 512], f32, tag="mC", name="pC")
                for g in range(G):
                    lhsT = zT_all[:, t, 256 * g : 256 * (g + 1)].rearrange(
                        "p (j i) -> p j i", j=2
                    )
                    fl = g == 0
                    ll = g == G - 1
                    wg = W_sb[:, 2 * g : 2 * g + 2, :]
                    nc.tensor.matmul(
                        pA, lhsT, wg[:, :, 0:512],
                        start=fl, stop=ll, perf_mode=DR,
                    )
                    nc.tensor.matmul(
                        pB, lhsT, wg[:, :, 512:1024],
                        start=fl, stop=ll, perf_mode=DR,
                    )
                    nc.tensor.matmul(
                        pC[:, 0:CK], lhsT, wg[:, :, 1024:NW],
                        start=fl, stop=ll, perf_mode=DR,
                    )
                return pA, pB, pC

            def emit_copies(t, pA, pB, pC):
                rstd = rstds[t]
                # z_sh[0:512] on scalar (gpsimd cannot read PSUM)
                nc.scalar.activation(
                    out=zshsp[:, t, 0:512], in_=pA, func=AF.Copy, scale=rstd
                )
                # z_sh[512:768] + z_sp in one vector op
                nc.vector.tensor_scalar_mul(
                    zshsp[:, t, 512:1024], pB, rstd
                )
                # sims on vector
                nc.vector.tensor_scalar_mul(
                    sraw_all[:, t, :], pC[:, 0:CK], rstd
                )
                if with_bias:
                    nc.vector.tensor_tensor(
                        out=zshsp[:, t, :], in0=zshsp[:, t, :],
                        in1=br_sb[0:1, 0:1024].partition_broadcast(P),
                        op=ALU.add,
                    )
                    nc.vector.tensor_tensor(
                        out=sraw_all[:, t, :], in0=sraw_all[:, t, :],
                        in1=br_sb[0:1, 1024:NW].partition_broadcast(P),
                        op=ALU.add,
                    )
                # squares: sq = 9 z_sh^2, accum gives the row norms
                nc.scalar.activation(
                    out=sq_all[:, t, :], in_=zshsp[:, t, 0:768],
                    func=AF.Square, bias=zero_t, scale=SQ_S,
                    accum_out=n2_all[:, t : t + 1],
                )

            accA = pacc.tile([P, 512], f32)
            accB = pacc.tile([P, 512], f32)

            def emit_ssq(pr):
                t2 = 2 * pr
                first = pr == 0
                last = pr == NPR - 1
                nc.tensor.matmul(
                    accA[0:32, 0:512], ones32,
                    sq_all[:, t2 : t2 + 2, 0:512],
                    start=first, stop=last,
                    perf_mode=DR, skip_group_check=True,
                )
                nc.tensor.matmul(
                    accB[0:32, 0:256], ones32,
                    sq_all[:, t2 : t2 + 2, 512:768],
                    start=first, stop=last,
                    perf_mode=DR, skip_group_check=True,
                )
                nc.tensor.matmul(
                    accB[0:32, 256:512], ones32,
                    zshsp[:, t2 : t2 + 2, 768:1024],
                    start=False, stop=last,
                    perf_mode=DR, skip_group_check=True,
                )

            def emit_chunk(ch):
                ts4 = slice(CHT * ch, CHT * (ch + 1))
                nrm = cpool.tile([P, CHT], f32, name="nrm")
                nc.scalar.activation(
                    out=nrm, in_=n2_all[:, ts4], func=AF.Sqrt,
                    bias=zero_t, scale=NRM_S,
                )
                rn = cpool.tile([P, CHT], f32, name="rn")
                nc.vector.reciprocal(out=rn, in_=nrm)
                sim = cpool.tile([P, CHT, CK], f32, name="simc")
                nc.gpsimd.tensor_tensor(
                    out=sim, in0=sraw_all[:, ts4, :],
                    in1=rn[:, :, None].to_broadcast([P, CHT, CK]),
                    op=ALU.mult,
                )
                t47 = cpool.tile([P, CHT, K, C], f32, name="t47")
                nc.vector.tensor_tensor(
                    out=t47,
                    in0=sim.rearrange("p t (c k) -> p t k c", k=K),
                    in1=mask_all[:, ts4, None, 0:C].to_broadcast([P, CHT, K, C]),
                    op=ALU.mult,
                )
                simK = cpool.tile([P, CHT, K], f32, name="simK")
                nc.vector.reduce_sum(out=simK, in_=t47, axis=AX)
                mx = cpool.tile([P, CHT], f32, name="mx")
                nc.vector.reduce_max(out=mx, in_=simK, axis=AX)
                dsub = cpool.tile([P, CHT, K], f32, name="dsub")
                nc.gpsimd.tensor_tensor(
                    out=dsub, in0=simK,
                    in1=mx[:, :, None].to_broadcast([P, CHT, K]),
                    op=ALU.subtract,
                )
                e = cpool.tile([P, CHT, K], f32, name="e")
                nc.scalar.activation(
                    out=e, in_=dsub, func=AF.Exp, scale=1.0 / TAU, bias=zero_t
                )
                se = cpool.tile([P, CHT], f32, name="se")
                nc.vector.reduce_sum(out=se, in_=e, axis=AX)
                rse = cpool.tile([P, CHT], f32, name="rse")
                nc.vector.reciprocal(out=rse, in_=se)
                q = cpool.tile([P, CHT, K], f32, name="q")
                nc.vector.tensor_tensor(
                    out=q, in0=e,
                    in1=rse[:, :, None].to_broadcast([P, CHT, K]),
                    op=ALU.mult,
                )
                nc.scalar.copy(out=R_all[:, ts4, 0:4], in_=q)
                lg = cpool.tile([P, CHT, K], f32, name="lg")
                nc.scalar.activation(out=lg, in_=q, func=AF.Ln, bias=eps8_t)
                ql = cpool.tile([P, CHT, K], f32, name="ql")
                nc.gpsimd.tensor_tensor(out=ql, in0=q, in1=lg, op=ALU.mult)
                qls = cpool.tile([P, CHT], f32, name="qls")
                nc.vector.reduce_sum(out=qls, in_=ql, axis=AX)
                nc.gpsimd.tensor_copy(
                    out=R_all[:, ts4, 4:5], in_=qls[:, :, None]
                )
                srl = cpool.tile([P, CHT, K], f32, name="srl")
                nc.gpsimd.tensor_tensor(
                    out=srl, in0=simK, in1=rlab_all[:, ts4, :], op=ALU.add
                )
                qsr = cpool.tile([P, CHT, K], f32, name="qsr")
                nc.gpsimd.tensor_tensor(out=qsr, in0=q, in1=srl, op=ALU.mult)
                s = cpool.tile([P, CHT], f32, name="s")
                nc.vector.reduce_sum(out=s, in_=qsr, axis=AX)
                # relu(dist_w - r_w) = Relu(1 - sum q*(simK + r))
                nc.scalar.activation(
                    out=strip_all[:, ts4], in_=s, func=AF.Relu,
                    scale=-1.0, bias=one_t,
                )

            # ---- main loop (stats 2 tiles ahead) ----
            for t in range(NT):
                if t == 0:
                    emit_stats(0, zr_tiles[0])
                    emit_stats(1, zr_tiles[1])
                if t + 2 < NT:
                    emit_stats(t + 2, zr_tiles[t + 2])
                mm = emit_mm(t)
                emit_copies(t, *mm)
                if t % 2 == 1 and t >= 3:
                    emit_ssq((t - 3) // 2)
                if t in (6, 10, 14):
                    emit_chunk((t - 6) // 4)
            emit_ssq(NPR - 1)

            # ---- corr + gram tails (reuse freed pA/pB banks) ----
            corr_done = []
            for jc in range(2):
                corrA = pA_pool.tile([P, 512], f32, tag="mA", name="corrA")
                corrBG = pB_pool.tile([P, 512], f32, tag="mB", name="corrBG")
                for pr in range(NPR):
                    t2 = 2 * pr
                    statn = zshsp[:, t2 : t2 + 2, 768 + jc * P : 768 + (jc + 1) * P]
                    nc.tensor.matmul(
                        corrA, statn, zshsp[:, t2 : t2 + 2, 0:512],
                        start=(pr == 0), stop=(pr == NPR - 1),
                        perf_mode=DR, skip_group_check=True,
                    )
                    nc.tensor.matmul(
                        corrBG[:, 0:256], statn, zshsp[:, t2 : t2 + 2, 512:768],
                        start=(pr == 0), stop=(pr == NPR - 1),
                        perf_mode=DR, skip_group_check=True,
                    )
                    nc.tensor.matmul(
                        corrBG[:, 256:512], statn, zshsp[:, t2 : t2 + 2, 768:1024],
                        start=False, stop=(pr == NPR - 1),
                        perf_mode=DR, skip_group_check=True,
                    )
                if jc == 0:
                    emit_chunk(3)
                corr_sb = outst.tile([P, DSH], b16, tag="corr_sb", name="corr_sb")
                nc.scalar.copy(out=corr_sb[:, 0:512], in_=corrA)
                nc.vector.tensor_copy(out=corr_sb[:, 512:768], in_=corrBG[:, 0:256])
                gram_sb = outst.tile([P, DSP], b16, tag="gram_sb", name="gram_sb")
                nc.vector.tensor_copy(out=gram_sb, in_=corrBG[:, 256:512])
                nc.sync.dma_start(
                    out=o_corr[jc * P : (jc + 1) * P, :], in_=corr_sb
                )
                nc.gpsimd.dma_start(
                    out=o_gram[jc * P : (jc + 1) * P, :], in_=gram_sb
                )

            # ---- segment-sum matmuls (bf16) ----
            segacc = pC_pool.tile([8, 512], f32, tag="mC", name="segacc")[:, 0:6]
            for t in range(NT):
                nc.tensor.matmul(
                    segacc, mask_bf[:, t, :], R_all[:, t, :],
                    start=(t == 0), stop=(t == NT - 1),
                    skip_group_check=True,
                )
            seg_sb = outst.tile([8, 6], f32, tag="seg_sb", name="seg_sb")
            nc.scalar.copy(out=seg_sb, in_=segacc)
            nc.gpsimd.dma_start(out=o_seg[:], in_=seg_sb)

            # ---- epilogue: accumulators + strip -> DRAM ----
            stat_sb = outst.tile([1, 1024], f32, tag="stat_sb", name="stat_sb")
            nc.scalar.copy(out=stat_sb[0:1, 0:512], in_=accA[0:1, 0:512])
            nc.vector.tensor_copy(
                out=stat_sb[0:1, 512:768], in_=accB[0:1, 0:256]
            )
            nc.scalar.copy(
                out=stat_sb[0:1, 768:1024], in_=accB[0:1, 256:512]
            )
            nc.gpsimd.dma_start(out=o_stat[:], in_=stat_sb)
            nc.sync.dma_start(out=o_intra[:], in_=strip_all)

    return _split_multiwaits(nc)


def _host_prep(inputs):
    import ml_dtypes

    bf16 = ml_dtypes.bfloat16
    fp8 = ml_dtypes.float8_e4m3
    z = np.asarray(inputs["z"], dtype=np.float32)
    labels = np.asarray(inputs["labels"]).astype(np.int64)
    gamma = np.asarray(inputs["ln_gamma"], dtype=np.float32)
    beta = np.asarray(inputs["ln_beta"], dtype=np.float32)
    W_sh = np.asarray(inputs["W_sh"], dtype=np.float32)
    b_sh = np.asarray(inputs["b_sh"], dtype=np.float32)
    W_sp = np.asarray(inputs["W_sp"], dtype=np.float32)
    b_sp = np.asarray(inputs["b_sp"], dtype=np.float32)
    centers = np.asarray(inputs["centers"], dtype=np.float32)
    radii = np.asarray(inputs["ema_radii"], dtype=np.float32)

    cf = centers.reshape(CK, DSH)
    cn = cf / np.maximum(
        np.linalg.norm(cf, axis=1, keepdims=True), 1e-12
    ).astype(np.float32)
    W_all = np.concatenate([W_sh, W_sp, W_sh @ cn.T], axis=1)  # [ZD, NW]
    W_eff = (gamma[:, None] * W_all).astype(np.float32)
    # fp8 feed: [p, g, j, col] with d = (2g + j)*128 + p
    wq = np.clip(W_eff * S_W, -240, 240).astype(fp8)
    w_feed = np.ascontiguousarray(
        wq.reshape(G, 2, P, NW).transpose(2, 0, 1, 3).reshape(P, 2 * G * NW)
    )

    be = beta @ W_all + np.concatenate([b_sh, b_sp, b_sh @ cn.T])
    b_eff = (S_H * be).astype(np.float32)
    with_bias = bool(np.any(b_eff != 0.0))

    # center z rows (the projection's rank-1 mean term, folded on host)
    zc = z - z.mean(axis=1, keepdims=True)
    zq = np.clip(zc, -240, 240).astype(fp8)
    zb = zc.astype(bf16)

    onehot = (labels[:, None] == np.arange(8)[None, :]).astype(np.float32)
    rlab = radii.reshape(C, K)[labels].astype(np.float32)  # [B, K]

    in_maps = []
    for i in range(NCORES):
        sl = slice(i * BL, (i + 1) * BL)
        # zt[t, p, kc*128 + i] = zq[t*128 + i, kc*128 + p]
        zt = (
            zq[sl]
            .reshape(NT, P, KC, P)
            .transpose(0, 3, 2, 1)
            .reshape(NT, P, KC * P)
        )
        m = {
            "zt": np.ascontiguousarray(zt),
            "zr": np.ascontiguousarray(zb[sl]),
            "w": w_feed,
            "mk": np.ascontiguousarray(onehot[sl]),
            "rl": np.ascontiguousarray(rlab[sl]),
        }
        if with_bias:
            m["br"] = np.ascontiguousarray(b_eff[None, :])
        in_maps.append(m)
    return in_maps, with_bias, cn


def _host_finish(results, cn):
    f64 = np.float64
    corr_raw = np.zeros((DSP, DSH), f64)
    gram = np.zeros((DSP, DSP), f64)
    stat = np.zeros(1024, f64)
    seg = np.zeros((8, 6), f64)
    intra_sum = 0.0
    for r in results:
        corr_raw += np.asarray(r["o_corr"]).astype(f64)
        gram += np.asarray(r["o_gram"]).astype(f64)
        stat += np.asarray(r["o_stat"])[0].astype(f64)
        seg += np.asarray(r["o_seg"]).astype(f64)
        intra_sum += float(np.asarray(r["o_intra"]).astype(f64).sum())

    ssq_sh = np.concatenate([stat[0:512], stat[512:768]]) / 9.0
    ssq_sp = np.diag(gram) / (S_H * S_H)
    sum_sp = stat[768:1024] / S_H
    corr_raw = corr_raw / (S_H * S_H)   # [DSP, DSH] = z_sp^T z_sh

    sum_q = seg[0:C, 0:4]
    qlsum_c = seg[0:C, 4]
    counts = seg[0:C, 5]

    n_sh = np.maximum(np.sqrt(ssq_sh), 1e-12)
    n_sp = np.maximum(np.sqrt(ssq_sp), 1e-12)
    corr = corr_raw.T / np.outer(n_sh, n_sp)
    L_ortho = (corr**2).mean()

    v = ssq_sp / B - (sum_sp / B) ** 2
    L_var = np.maximum(0.05 - v, 0.0).mean()

    L_intra = intra_sum / B

    p = sum_q / (sum_q.sum(-1, keepdims=True) + 1e-8)
    H_marg = -(p * np.log(p + 1e-8)).sum(-1)
    H_cond = (-qlsum_c) / np.maximum(counts, 1.0)
    valid = counts > 0
    L_bal_k = np.log(f64(K)) - H_marg + H_cond
    L_balance = np.where(valid, L_bal_k, 0.0).sum() / max(int(valid.sum()), 1)

    sim_mat = (cn @ cn.T).astype(f64)
    blkmask = 1.0 - np.kron(np.eye(C), np.ones((K, K)))
    L_overlap = (np.maximum(sim_mat - 0.3, 0.0) * blkmask).sum() / (
        blkmask.sum() + 1e-6
    )
    cnr = cn.reshape(C, K, DSH).astype(f64)
    sims_in = np.einsum("ckd,cld->ckl", cnr, cnr)
    triu = np.triu(np.ones((K, K)), 1)
    L_div = (np.maximum(sims_in - 0.8, 0.0) * triu).sum() / max(
        C * K * (K - 1) // 2, 1
    )

    L_ball = L_intra + 0.3 * L_overlap + 0.2 * L_div + 0.15 * L_balance
    loss = L_ball + 0.02 * L_ortho + 0.005 * L_var
    return np.float32(loss)


def _run_hw(nc, in_maps, trace=False, tmpdir=None):
    from concourse.bass_utils import run_bass_kernel_spmd

    res = run_bass_kernel_spmd(
        nc, in_maps, core_ids=list(range(NCORES)), trace=trace, tmpdir=tmpdir
    )
    return res


def _run_sim(nc, in_maps):
    from concourse.bass_interp import CoreSim

    outs = []
    for i, im in enumerate(in_maps):
        sim = CoreSim(nc, publish_trace=False)
        sim.assign_tensors(im)
        sim.simulate()
        outs.append(
            {k: np.array(sim.tensor(k)) for k in
             ("o_corr", "o_gram", "o_stat", "o_seg", "o_intra")}
        )
    return outs


def kernel(**inputs) -> np.ndarray:
    in_maps, with_bias, cn = _host_prep(inputs)
    if with_bias not in _GRAPH_CACHE:
        _GRAPH_CACHE[with_bias] = _build_graph(with_bias)
    nc = _GRAPH_CACHE[with_bias]
    if os.environ.get("KERNEL_BASS_SIM"):
        results = _run_sim(nc, in_maps)
    else:
        results = _run_hw(nc, in_maps).results
    return _host_finish(results, cn)


# revision 31
# speedup vs baseline: 1.5948x; 1.0563x over previous
"""Trainium2 Bass kernel for nn_AngularMultiCenterEmotionBall.

Data-parallel over batch B=16384 across 8 NeuronCores (2048 rows/core).

The projection GEMM z0 @ [W_sh | W_sp | W_sh @ c_norm.T] runs in fp8
(e4m3) with MatmulPerfMode.DoubleRow: each matmul instruction contracts
2x128 rows, doubling PE throughput vs bf16.  The host centers z
(mean-subtract, a shift the projection is equivariant to) and the
per-row 1/std LayerNorm scale is computed on-device via bn_stats and
folded into the PSUM->SBUF copies.  gamma/beta are folded into the
projection weights on the host (beta == 0 here).

Per-core device work:
  - bn_stats over centered-z rows -> rstd (the only LN stat left)
  - fp8 DoubleRow GEMM  (zc/std) @ [W_sh | W_sp | W_sh cnT] * scales
  - z_sh/z_sp stored fp8 at 16x natural scale; squares at 9x/8x via
    one ACT Square (with accum_out row-norms) and one Pool STT
  - column sums-of-squares / sums via ones-stationary DoubleRow
    matmuls packed into ONE PSUM bank at out partitions {0,32,64}
    (bank pre-cleared by a zeros-stationary matmul)
  - per-sample softmax q over the label's 4 centers, relu(dist-r)
  - segment stats (sum_q, sum q log q, counts) via one-hot matmuls
  - cross-correlation z_sp.T @ [z_sh | z_sp] in fp8 DoubleRow (the
    z_sp Gram diagonal supplies ssq_sp for the variance-floor loss)
The host sums the 8 partial outputs and finishes the scalar math
(plus the centers-only overlap/diversity losses).
"""

import os
import sys

import numpy as np

sys.path.insert(0, "/opt/trn_rl_repo")

# problem constants (hardcoded per harness contract)
B, ZD, C, K = 16384, 1024, 7, 4
DSH, DSP = 768, 256
TAU = 0.15
NCORES = 8
BL = B // NCORES          # 2048 rows per core
P = 128
NT = BL // P              # 16 row-tiles per core
CK = C * K                # 28
NW = DSH + DSP + CK       # 1052 fused output columns
KC = ZD // P              # 8 contraction chunks
G = KC // 2               # 4 DoubleRow groups (256 contraction each)
NPR = NT // 2             # 8 tile pairs
CHT = 4                   # tiles per softmax chunk
NCH = NT // CHT           # 4 chunks

S_W = 64.0                # fp8 weight scale
S_H = 16.0                # fp8 z_sh/z_sp storage scale
SQ_S = 0.1875             # ACT Square input scale: sqh = 9 * z_sh^2
SQP_S = 1.0 / 32.0        # Pool STT scale:        sqp = 8 * z_sp^2
NRM_S = (S_H * S_H) / (S_H * SQ_S) ** 2   # 256/9: nrm = 16*||z_sh||

_GRAPH_CACHE = {}


def _split_multiwaits(nc):
    """Walrus codegen in this container accepts at most one semaphore wait
    per engine instruction. TileContext attaches several. Peel the extra
    waits off into standalone single-wait EventSemaphore instructions
    (what raw-bass wait_ge emits) placed just before the instruction —
    the engine is in-order, so wait(A); wait(B); op == op waiting {A,B}.
    Applied as a JSON rewrite at serialization time."""
    import json

    orig = nc.to_json_bytes

    def patched():
        d = json.loads(orig())
        ctr = [0]
        for f in d["functions"]:
            for b in f["blocks"]:
                insts = b.get("instructions")
                if not insts:
                    continue
                out = []
                for i in insts:
                    si = i.get("sync_info") or {}
                    waits = si.get("on_wait") or []
                    if len(waits) > 1:
                        for w in waits[:-1]:
                            ctr[0] += 1
                            out.append(
                                {
                                    "engine": i["engine"],
                                    "ins": [],
                                    "name": f"splitwait_{ctr[0]}",
                                    "opcode": "EventSemaphore",
                                    "outs": [],
                                    "sync_info": {
                                        "on_update": [],
                                        "on_wait": [w],
                                    },
                                }
                            )
                        si["on_wait"] = [waits[-1]]
                    out.append(i)
                b["instructions"] = out
        return json.dumps(d).encode()

    nc.to_json_bytes = patched
    return nc


def _build_graph(with_bias: bool):
    import concourse.bass as bass
    import concourse.tile as tile
    from concourse import mybir

    f32 = mybir.dt.float32
    b16 = mybir.dt.bfloat16
    f8 = mybir.dt.float8e4
    AF = mybir.ActivationFunctionType
    ALU = mybir.AluOpType
    DR = mybir.MatmulPerfMode.DoubleRow
    AX = mybir.AxisListType.X

    nc = bass.Bass()
    zt_ext = nc.declare_dram_parameter("zt", [NT, P, KC * P], f8, isOutput=False)
    zr_ext = nc.declare_dram_parameter("zr", [BL, ZD], b16, isOutput=False)
    w_ext = nc.declare_dram_parameter("w", [P, 2 * G * NW], f8, isOutput=False)
    mk_ext = nc.declare_dram_parameter("mk", [BL, 8], f32, isOutput=False)
    rl_ext = nc.declare_dram_parameter("rl", [BL, K], f32, isOutput=False)
    if with_bias:
        br_ext = nc.declare_dram_parameter("br", [1, NW], f32, isOutput=False)
    o_corr = nc.declare_dram_parameter("o_corr", [DSP, DSH], b16, isOutput=True)
    o_gram = nc.declare_dram_parameter("o_gram", [DSP, DSP], b16, isOutput=True)
    o_stat = nc.declare_dram_parameter("o_stat", [1, 1024], f32, isOutput=True)
    o_seg = nc.declare_dram_parameter("o_seg", [8, 6], f32, isOutput=True)
    o_intra = nc.declare_dram_parameter("o_intra", [P, NT], f32, isOutput=True)

    with tile.TileContext(nc) as tc:
        with (
            tc.tile_pool(name="singles", bufs=1) as singles,

            tc.tile_pool(name="stats", bufs=6) as stats,
            tc.tile_pool(name="cpool", bufs=2) as cpool,
            tc.tile_pool(name="outst", bufs=2) as outst,
            tc.tile_pool(name="pA", bufs=2, space="PSUM") as pA_pool,
            tc.tile_pool(name="pB", bufs=2, space="PSUM") as pB_pool,
            tc.tile_pool(name="pC", bufs=2, space="PSUM") as pC_pool,
            tc.tile_pool(name="pacc", bufs=1, space="PSUM") as pacc,
        ):
            # ---- persistent SBUF state ----
            W_sb = singles.tile([P, 2 * G, NW], f8)
            zT_all = singles.tile([P, NT, KC * P], f8)
            zshsp = singles.tile([P, NT, 1024], f8)   # [z_sh 768 | z_sp 256]
            sq_all = singles.tile([P, NT, DSH], f8)   # 9 * z_sh^2
            sraw_all = singles.tile([P, NT, CK], f32)  # 16x natural sims
            n2_all = singles.tile([P, NT], f32)
            mask_all = singles.tile([P, NT, 8], f32)
            mask_bf = singles.tile([P, NT, 8], b16)
            rlab_all = singles.tile([P, NT, K], f32)
            R_all = singles.tile([P, NT, 6], b16)
            strip_all = singles.tile([P, NT], f32)

            # ---- input DMAs: few big transfers (SWDGE gen is ~1us each) ----
            zr_all = singles.tile([P, NT, 2, 512], b16)
            mv_all = singles.tile([P, NT, 2], f32)

            # sync queue: z-transposed tiles in 4-tile chunks, then masks
            for c in range(0, NT, 4):
                nc.sync.dma_start(
                    out=zT_all[:, c : c + 4, :],
                    in_=zt_ext[c : c + 4].rearrange("t p c -> p t c"),
                )
            nc.sync.dma_start(
                out=mask_all, in_=mk_ext[:].rearrange("(t p) c -> p t c", p=P)
            )
            nc.sync.dma_start(
                out=rlab_all, in_=rl_ext[:].rearrange("(t p) k -> p t k", p=P)
            )
            # scalar queue: all weights in one transfer
            nc.scalar.dma_start(
                out=W_sb, in_=w_ext[:].rearrange("p (g c) -> p g c", g=2 * G)
            )
            # gpsimd queue: row-major z, front-loaded for the stats pipeline
            for c0, c1 in ((0, 2), (2, 6), (6, 10), (10, 14), (14, 16)):
                nc.gpsimd.dma_start(
                    out=zr_all[:, c0:c1, :, :],
                    in_=zr_ext[c0 * P : c1 * P, :].rearrange(
                        "(t b) (g f) -> b t g f", b=P, g=2
                    ),
                )
            if with_bias:
                br_sb = singles.tile([1, NW], f32)
                nc.vector.dma_start(out=br_sb, in_=br_ext[:])

            # constants
            zero_t = singles.tile([P, 1], f32)
            nc.gpsimd.memset(zero_t, 0.0)
            one_t = singles.tile([P, 1], f32)
            nc.gpsimd.memset(one_t, 1.0)
            eps8_t = singles.tile([P, 1], f32)
            nc.gpsimd.memset(eps8_t, 1e-8)
            seps_t = singles.tile([P, 1], f32)
            nc.gpsimd.memset(seps_t, (S_W / S_H) ** 2 * 1e-5)
            # ones-in-column-0 stationary: PE stationary tiles are 32-wide
            # minimum, and DoubleRow only works at tile position (0, 0), so
            # each stat sum lands on out partitions 0:32 with row 0 live.
            ones32 = singles.tile([P, 2, 32], f8)
            nc.gpsimd.memset(ones32, 0.0)
            nc.scalar.copy(
                out=ones32[:, :, 0:1],
                in_=one_t[:, None, 0:1].to_broadcast([P, 2, 1]),
            )

            rstds = [None] * NT

            def emit_stats(t):
                st = stats.tile([P, 2, 6], b16, name="st")
                nc.vector.bn_stats(out=st[:, 0, :], in_=zr_all[:, t, 0, :])
                nc.vector.bn_stats(out=st[:, 1, :], in_=zr_all[:, t, 1, :])
                nc.vector.bn_aggr(out=mv_all[:, t, :], in_=st)

            def emit_rstd4(c):
                # rstd_eff = S_H/(S_W*std) = exp(-0.5*ln(c16*(var+eps)))
                # computed via Ln+Exp so the ACT engine never needs the
                # sqrt table set (keeps one act table resident all kernel)
                ts4 = slice(CHT * c, CHT * (c + 1))
                lnv = stats.tile([P, CHT], f32, name="lnv")
                nc.scalar.activation(
                    out=lnv[:, :, None], in_=mv_all[:, ts4, 1:2],
                    func=AF.Ln, bias=seps_t, scale=(S_W / S_H) ** 2,
                )
                rstd4 = stats.tile([P, CHT], f32, name="rstd4")
                nc.scalar.activation(
                    out=rstd4, in_=lnv, func=AF.Exp, scale=-0.5, bias=zero_t
                )
                for i in range(CHT):
                    rstds[CHT * c + i] = rstd4[:, i : i + 1]

            def emit_mm(t):
                pA = pA_pool.tile([P, 512], f32, tag="mA", name="pA")
                pB = pB_pool.tile([P, 512], f32, tag="mB", name="pB")
                # full-bank tile: a sub-bank tile would share its bank with
                # the other buf, and start=True clears has_written bank-wide
                pC = pC_pool.tile([P, 512], f32, tag="mC", name="pC")
                for g in range(G):
                    lhsT = zT_all[:, t, 256 * g : 256 * (g + 1)].rearrange(
                        "p (j i) -> p j i", j=2
                    )
                    fl = g == 0
                    ll = g == G - 1
                    wg = W_sb[:, 2 * g : 2 * g + 2, :]
                    nc.tensor.matmul(
                        pA, lhsT, wg[:, :, 0:512],
                        start=fl, stop=ll, perf_mode=DR,
                    )
                    nc.tensor.matmul(
                        pB, lhsT, wg[:, :, 512:1024],
                        start=fl, stop=ll, perf_mode=DR,
                    )
                    nc.tensor.matmul(
                        pC[:, 0:CK], lhsT, wg[:, :, 1024:NW],
                        start=fl, stop=ll, perf_mode=DR,
                    )
                return pA, pB, pC

            def emit_copies(t, pA, pB, pC):
                rstd = rstds[t]
                # z_sh[0:512] on scalar (gpsimd cannot read PSUM)
                nc.scalar.activation(
                    out=zshsp[:, t, 0:512], in_=pA, func=AF.Copy, scale=rstd
                )
                # z_sh[512:768] + z_sp in one vector op
                nc.vector.tensor_scalar_mul(
                    zshsp[:, t, 512:1024], pB, rstd
                )
                # sims on scalar
                nc.scalar.activation(
                    out=sraw_all[:, t, :], in_=pC[:, 0:CK],
                    func=AF.Copy, scale=rstd,
                )
                if with_bias:
                    nc.vector.tensor_tensor(
                        out=zshsp[:, t, :], in0=zshsp[:, t, :],
                        in1=br_sb[0:1, 0:1024].partition_broadcast(P),
                        op=ALU.add,
                    )
                    nc.vector.tensor_tensor(
                        out=sraw_all[:, t, :], in0=sraw_all[:, t, :],
                        in1=br_sb[0:1, 1024:NW].partition_broadcast(P),
                        op=ALU.add,
                    )
                # squares: sq = 9 z_sh^2, accum gives the row norms
                nc.scalar.activation(
                    out=sq_all[:, t, :], in_=zshsp[:, t, 0:768],
                    func=AF.Square, bias=zero_t, scale=SQ_S,
                    accum_out=n2_all[:, t : t + 1],
                )

            accA = pacc.tile([P, 512], f32)
            accB = pacc.tile([P, 512], f32)

            def emit_ssq(pr):
                t2 = 2 * pr
                first = pr == 0
                last = pr == NPR - 1
                nc.tensor.matmul(
                    accA[0:32, 0:512], ones32,
                    sq_all[:, t2 : t2 + 2, 0:512],
                    start=first, stop=last,
                    perf_mode=DR, skip_group_check=True,
                )
                nc.tensor.matmul(
                    accB[0:32, 0:256], ones32,
                    sq_all[:, t2 : t2 + 2, 512:768],
                    start=first, stop=last,
                    perf_mode=DR, skip_group_check=True,
                )
                nc.tensor.matmul(
                    accB[0:32, 256:512], ones32,
                    zshsp[:, t2 : t2 + 2, 768:1024],
                    start=False, stop=last,
                    perf_mode=DR, skip_group_check=True,
                )

            def emit_chunk(ch):
                ts4 = slice(CHT * ch, CHT * (ch + 1))
                # rn = 1/(16*||z_sh||) = exp(-0.5*ln(n2*NRM_S)), sqrt-free
                lnn = cpool.tile([P, CHT], f32, name="lnn")
                nc.scalar.activation(
                    out=lnn, in_=n2_all[:, ts4], func=AF.Ln,
                    bias=eps8_t, scale=NRM_S,
                )
                rn = cpool.tile([P, CHT], f32, name="rn")
                nc.scalar.activation(
                    out=rn, in_=lnn, func=AF.Exp, scale=-0.5, bias=zero_t
                )
                sim = cpool.tile([P, CHT, CK], f32, name="simc")
                nc.gpsimd.tensor_tensor(
                    out=sim, in0=sraw_all[:, ts4, :],
                    in1=rn[:, :, None].to_broadcast([P, CHT, CK]),
                    op=ALU.mult,
                )
                t47 = cpool.tile([P, CHT, K, C], f32, name="t47")
                nc.vector.tensor_tensor(
                    out=t47,
                    in0=sim.rearrange("p t (c k) -> p t k c", k=K),
                    in1=mask_all[:, ts4, None, 0:C].to_broadcast([P, CHT, K, C]),
                    op=ALU.mult,
                )
                simK = cpool.tile([P, CHT, K], f32, name="simK")
                nc.vector.reduce_sum(out=simK, in_=t47, axis=AX)
                mx = cpool.tile([P, CHT], f32, name="mx")
                nc.vector.reduce_max(out=mx, in_=simK, axis=AX)
                dsub = cpool.tile([P, CHT, K], f32, name="dsub")
                nc.gpsimd.tensor_tensor(
                    out=dsub, in0=simK,
                    in1=mx[:, :, None].to_broadcast([P, CHT, K]),
                    op=ALU.subtract,
                )
                e = cpool.tile([P, CHT, K], f32, name="e")
                nc.scalar.activation(
                    out=e, in_=dsub, func=AF.Exp, scale=1.0 / TAU, bias=zero_t
                )
                se = cpool.tile([P, CHT], f32, name="se")
                nc.vector.reduce_sum(out=se, in_=e, axis=AX)
                rse = cpool.tile([P, CHT], f32, name="rse")
                nc.vector.reciprocal(out=rse, in_=se)
                q = cpool.tile([P, CHT, K], f32, name="q")
                nc.gpsimd.tensor_tensor(
                    out=q, in0=e,
                    in1=rse[:, :, None].to_broadcast([P, CHT, K]),
                    op=ALU.mult,
                )
                nc.scalar.copy(out=R_all[:, ts4, 0:4], in_=q)
                lg = cpool.tile([P, CHT, K], f32, name="lg")
                nc.scalar.activation(out=lg, in_=q, func=AF.Ln, bias=eps8_t)
                ql = cpool.tile([P, CHT, K], f32, name="ql")
                nc.gpsimd.tensor_tensor(out=ql, in0=q, in1=lg, op=ALU.mult)
                qls = cpool.tile([P, CHT], f32, name="qls")
                nc.vector.reduce_sum(out=qls, in_=ql, axis=AX)
                nc.gpsimd.tensor_copy(
                    out=R_all[:, ts4, 4:5], in_=qls[:, :, None]
                )
                srl = cpool.tile([P, CHT, K], f32, name="srl")
                nc.gpsimd.tensor_tensor(
                    out=srl, in0=simK, in1=rlab_all[:, ts4, :], op=ALU.add
                )
                qsr = cpool.tile([P, CHT, K], f32, name="qsr")
                nc.gpsimd.tensor_tensor(out=qsr, in0=q, in1=srl, op=ALU.mult)
                s = cpool.tile([P, CHT], f32, name="s")
                nc.vector.reduce_sum(out=s, in_=qsr, axis=AX)
                # relu(dist_w - r_w) = Relu(1 - sum q*(simK + r))
                nc.scalar.activation(
                    out=strip_all[:, ts4], in_=s, func=AF.Relu,
                    scale=-1.0, bias=one_t,
                )

            # ---- main loop (stats 2 tiles ahead) ----
            for t in range(NT):
                if t == 0:
                    for s in range(5):
                        emit_stats(s)
                        if s % CHT == CHT - 1:
                            emit_rstd4(s // CHT)
                if t + 5 < NT:
                    s = t + 5
                    emit_stats(s)
                    if s % CHT == CHT - 1:
                        emit_rstd4(s // CHT)
                mm = emit_mm(t)
                emit_copies(t, *mm)
                if t % 2 == 1 and t >= 3:
                    emit_ssq((t - 3) // 2)
                if t in (6, 10, 14):
                    emit_chunk((t - 6) // 4)
            emit_ssq(NPR - 1)
            emit_chunk(3)
            # seg stationaries, deferred here so the early scalar stream
            # is not blocked waiting on the (late) mask DMA
            nc.scalar.copy(out=mask_bf, in_=mask_all)
            nc.scalar.copy(
                out=R_all[:, :, 5:6],
                in_=one_t[:, None, 0:1].to_broadcast([P, NT, 1]),
            )

            # ---- corr + gram tails (reuse freed pA/pB banks) ----
            corr_done = []
            for jc in range(2):
                corrA = pA_pool.tile([P, 512], f32, tag="mA", name="corrA")
                corrBG = pB_pool.tile([P, 512], f32, tag="mB", name="corrBG")
                for pr in range(NPR):
                    t2 = 2 * pr
                    statn = zshsp[:, t2 : t2 + 2, 768 + jc * P : 768 + (jc + 1) * P]
                    nc.tensor.matmul(
                        corrA, statn, zshsp[:, t2 : t2 + 2, 0:512],
                        start=(pr == 0), stop=(pr == NPR - 1),
                        perf_mode=DR, skip_group_check=True,
                    )
                    nc.tensor.matmul(
                        corrBG[:, 0:256], statn, zshsp[:, t2 : t2 + 2, 512:768],
                        start=(pr == 0), stop=(pr == NPR - 1),
                        perf_mode=DR, skip_group_check=True,
                    )
                    nc.tensor.matmul(
                        corrBG[:, 256:512], statn, zshsp[:, t2 : t2 + 2, 768:1024],
                        start=False, stop=(pr == NPR - 1),
                        perf_mode=DR, skip_group_check=True,
                    )
                corr_sb = outst.tile([P, DSH], b16, tag="corr_sb", name="corr_sb")
                nc.scalar.copy(out=corr_sb[:, 0:512], in_=corrA)
                nc.vector.tensor_copy(out=corr_sb[:, 512:768], in_=corrBG[:, 0:256])
                gram_sb = outst.tile([P, DSP], b16, tag="gram_sb", name="gram_sb")
                nc.vector.tensor_copy(out=gram_sb, in_=corrBG[:, 256:512])
                nc.sync.dma_start(
                    out=o_corr[jc * P : (jc + 1) * P, :], in_=corr_sb
                )
                nc.gpsimd.dma_start(
                    out=o_gram[jc * P : (jc + 1) * P, :], in_=gram_sb
                )

            # ---- segment-sum matmuls (bf16) ----
            segacc = pC_pool.tile([8, 512], f32, tag="mC", name="segacc")[:, 0:6]
            for t in range(NT):
                nc.tensor.matmul(
                    segacc, mask_bf[:, t, :], R_all[:, t, :],
                    start=(t == 0), stop=(t == NT - 1),
                    skip_group_check=True,
                )
            seg_sb = outst.tile([8, 6], f32, tag="seg_sb", name="seg_sb")
            nc.scalar.copy(out=seg_sb, in_=segacc)
            nc.gpsimd.dma_start(out=o_seg[:], in_=seg_sb)

            # ---- epilogue: accumulators + strip -> DRAM ----
            stat_sb = outst.tile([1, 1024], f32, tag="stat_sb", name="stat_sb")
            nc.scalar.copy(out=stat_sb[0:1, 0:512], in_=accA[0:1, 0:512])
            nc.vector.tensor_copy(
                out=stat_sb[0:1, 512:768], in_=accB[0:1, 0:256]
            )
            nc.scalar.copy(
                out=stat_sb[0:1, 768:1024], in_=accB[0:1, 256:512]
            )
            nc.gpsimd.dma_start(out=o_stat[:], in_=stat_sb)
            nc.sync.dma_start(out=o_intra[:], in_=strip_all)

    return _split_multiwaits(nc)


def _host_prep(inputs):
    import ml_dtypes

    bf16 = ml_dtypes.bfloat16
    fp8 = ml_dtypes.float8_e4m3
    z = np.asarray(inputs["z"], dtype=np.float32)
    labels = np.asarray(inputs["labels"]).astype(np.int64)
    gamma = np.asarray(inputs["ln_gamma"], dtype=np.float32)
    beta = np.asarray(inputs["ln_beta"], dtype=np.float32)
    W_sh = np.asarray(inputs["W_sh"], dtype=np.float32)
    b_sh = np.asarray(inputs["b_sh"], dtype=np.float32)
    W_sp = np.asarray(inputs["W_sp"], dtype=np.float32)
    b_sp = np.asarray(inputs["b_sp"], dtype=np.float32)
    centers = np.asarray(inputs["centers"], dtype=np.float32)
    radii = np.asarray(inputs["ema_radii"], dtype=np.float32)

    cf = centers.reshape(CK, DSH)
    cn = cf / np.maximum(
        np.linalg.norm(cf, axis=1, keepdims=True), 1e-12
    ).astype(np.float32)
    W_all = np.concatenate([W_sh, W_sp, W_sh @ cn.T], axis=1)  # [ZD, NW]
    W_eff = (gamma[:, None] * W_all).astype(np.float32)
    # fp8 feed: [p, g, j, col] with d = (2g + j)*128 + p
    wq = np.clip(W_eff * S_W, -240, 240).astype(fp8)
    w_feed = np.ascontiguousarray(
        wq.reshape(G, 2, P, NW).transpose(2, 0, 1, 3).reshape(P, 2 * G * NW)
    )

    be = beta @ W_all + np.concatenate([b_sh, b_sp, b_sh @ cn.T])
    b_eff = (S_H * be).astype(np.float32)
    with_bias = bool(np.any(b_eff != 0.0))

    # center z rows (the projection's rank-1 mean term, folded on host)
    zc = z - z.mean(axis=1, keepdims=True)
    zq = np.clip(zc, -240, 240).astype(fp8)
    zb = zc.astype(bf16)

    onehot = (labels[:, None] == np.arange(8)[None, :]).astype(np.float32)
    rlab = radii.reshape(C, K)[labels].astype(np.float32)  # [B, K]

    in_maps = []
    for i in range(NCORES):
        sl = slice(i * BL, (i + 1) * BL)
        # zt[t, p, kc*128 + i] = zq[t*128 + i, kc*128 + p]
        zt = (
            zq[sl]
            .reshape(NT, P, KC, P)
            .transpose(0, 3, 2, 1)
            .reshape(NT, P, KC * P)
        )
        m = {
            "zt": np.ascontiguousarray(zt),
            "zr": np.ascontiguousarray(zb[sl]),
            "w": w_feed,
            "mk": np.ascontiguousarray(onehot[sl]),
            "rl": np.ascontiguousarray(rlab[sl]),
        }
        if with_bias:
            m["br"] = np.ascontiguousarray(b_eff[None, :])
        in_maps.append(m)
    return in_maps, with_bias, cn


def _host_finish(results, cn):
    f64 = np.float64
    corr_raw = np.zeros((DSP, DSH), f64)
    gram = np.zeros((DSP, DSP), f64)
    stat = np.zeros(1024, f64)
    seg = np.zeros((8, 6), f64)
    intra_sum = 0.0
    for r in results:
        corr_raw += np.asarray(r["o_corr"]).astype(f64)
        gram += np.asarray(r["o_gram"]).astype(f64)
        stat += np.asarray(r["o_stat"])[0].astype(f64)
        seg += np.asarray(r["o_seg"]).astype(f64)
        intra_sum += float(np.asarray(r["o_intra"]).astype(f64).sum())

    ssq_sh = np.concatenate([stat[0:512], stat[512:768]]) / 9.0
    ssq_sp = np.diag(gram) / (S_H * S_H)
    sum_sp = stat[768:1024] / S_H
    corr_raw = corr_raw / (S_H * S_H)   # [DSP, DSH] = z_sp^T z_sh

    sum_q = seg[0:C, 0:4]
    qlsum_c = seg[0:C, 4]
    counts = seg[0:C, 5]

    n_sh = np.maximum(np.sqrt(ssq_sh), 1e-12)
    n_sp = np.maximum(np.sqrt(ssq_sp), 1e-12)
    corr = corr_raw.T / np.outer(n_sh, n_sp)
    L_ortho = (corr**2).mean()

    v = ssq_sp / B - (sum_sp / B) ** 2
    L_var = np.maximum(0.05 - v, 0.0).mean()

    L_intra = intra_sum / B

    p = sum_q / (sum_q.sum(-1, keepdims=True) + 1e-8)
    H_marg = -(p * np.log(p + 1e-8)).sum(-1)
    H_cond = (-qlsum_c) / np.maximum(counts, 1.0)
    valid = counts > 0
    L_bal_k = np.log(f64(K)) - H_marg + H_cond
    L_balance = np.where(valid, L_bal_k, 0.0).sum() / max(int(valid.sum()), 1)

    sim_mat = (cn @ cn.T).astype(f64)
    blkmask = 1.0 - np.kron(np.eye(C), np.ones((K, K)))
    L_overlap = (np.maximum(sim_mat - 0.3, 0.0) * blkmask).sum() / (
        blkmask.sum() + 1e-6
    )
    cnr = cn.reshape(C, K, DSH).astype(f64)
    sims_in = np.einsum("ckd,cld->ckl", cnr, cnr)
    triu = np.triu(np.ones((K, K)), 1)
    L_div = (np.maximum(sims_in - 0.8, 0.0) * triu).sum() / max(
        C * K * (K - 1) // 2, 1
    )

    L_ball = L_intra + 0.3 * L_overlap + 0.2 * L_div + 0.15 * L_balance
    loss = L_ball + 0.02 * L_ortho + 0.005 * L_var
    return np.float32(loss)


def _run_hw(nc, in_maps, trace=False, tmpdir=None):
    from concourse.bass_utils import run_bass_kernel_spmd

    res = run_bass_kernel_spmd(
        nc, in_maps, core_ids=list(range(NCORES)), trace=trace, tmpdir=tmpdir
    )
    return res


def _run_sim(nc, in_maps):
    from concourse.bass_interp import CoreSim

    outs = []
    for i, im in enumerate(in_maps):
        sim = CoreSim(nc, publish_trace=False)
        sim.assign_tensors(im)
        sim.simulate()
        outs.append(
            {k: np.array(sim.tensor(k)) for k in
             ("o_corr", "o_gram", "o_stat", "o_seg", "o_intra")}
        )
    return outs


def kernel(**inputs) -> np.ndarray:
    in_maps, with_bias, cn = _host_prep(inputs)
    if with_bias not in _GRAPH_CACHE:
        _GRAPH_CACHE[with_bias] = _build_graph(with_bias)
    nc = _GRAPH_CACHE[with_bias]
    if os.environ.get("KERNEL_BASS_SIM"):
        results = _run_sim(nc, in_maps)
    else:
        results = _run_hw(nc, in_maps).results
    return _host_finish(results, cn)


# revision 37
# speedup vs baseline: 1.7605x; 1.1039x over previous
"""Trainium2 Bass kernel for nn_AngularMultiCenterEmotionBall.

Data-parallel over batch B=16384 across 8 NeuronCores (2048 rows/core).

The projection GEMM z0 @ [W_sh | W_sp | W_sh @ c_norm.T] runs in fp8
(e4m3) with MatmulPerfMode.DoubleRow: each matmul instruction contracts
2x128 rows, doubling PE throughput vs bf16.  The host centers z
(mean-subtract, a shift the projection is equivariant to) and the
per-row 1/std LayerNorm scale is computed on-device via bn_stats and
folded into the PSUM->SBUF copies.  gamma/beta are folded into the
projection weights on the host (beta == 0 here).

Per-core device work:
  - bn_stats over centered-z rows -> rstd (the only LN stat left)
  - fp8 DoubleRow GEMM  (zc/std) @ [W_sh | W_sp | W_sh cnT] * scales
  - z_sh/z_sp stored fp8 at 16x natural scale; squares at 9x/8x via
    one ACT Square (with accum_out row-norms) and one Pool STT
  - column sums-of-squares / sums via ones-stationary DoubleRow
    matmuls packed into ONE PSUM bank at out partitions {0,32,64}
    (bank pre-cleared by a zeros-stationary matmul)
  - per-sample softmax q over the label's 4 centers, relu(dist-r)
  - segment stats (sum_q, sum q log q, counts) via one-hot matmuls
  - cross-correlation z_sp.T @ [z_sh | z_sp] in fp8 DoubleRow (the
    z_sp Gram diagonal supplies ssq_sp for the variance-floor loss)
The host sums the 8 partial outputs and finishes the scalar math
(plus the centers-only overlap/diversity losses).
"""

import os
import sys

import numpy as np

sys.path.insert(0, "/opt/trn_rl_repo")

# problem constants (hardcoded per harness contract)
B, ZD, C, K = 16384, 1024, 7, 4
DSH, DSP = 768, 256
TAU = 0.15
NCORES = 8
BL = B // NCORES          # 2048 rows per core
P = 128
NT = BL // P              # 16 row-tiles per core
CK = C * K                # 28
NW = DSH + DSP + CK       # 1052 fused output columns
KC = ZD // P              # 8 contraction chunks
G = KC // 2               # 4 DoubleRow groups (256 contraction each)
NPR = NT // 2             # 8 tile pairs
CHT = 4                   # tiles per softmax chunk
NCH = NT // CHT           # 4 chunks

S_W = 64.0                # fp8 weight scale
S_H = 16.0                # fp8 z_sh/z_sp storage scale
SQ_S = 0.1875             # ACT Square input scale: sqh = 9 * z_sh^2
SQP_S = 1.0 / 32.0        # Pool STT scale:        sqp = 8 * z_sp^2
NRM_S = (S_H * S_H) / (S_H * SQ_S) ** 2   # 256/9: nrm = 16*||z_sh||

_GRAPH_CACHE = {}


def _split_multiwaits(nc):
    """Walrus codegen in this container accepts at most one semaphore wait
    per engine instruction. TileContext attaches several. Peel the extra
    waits off into standalone single-wait EventSemaphore instructions
    (what raw-bass wait_ge emits) placed just before the instruction —
    the engine is in-order, so wait(A); wait(B); op == op waiting {A,B}.
    Applied as a JSON rewrite at serialization time."""
    import json

    orig = nc.to_json_bytes

    def patched():
        d = json.loads(orig())
        ctr = [0]
        for f in d["functions"]:
            for b in f["blocks"]:
                insts = b.get("instructions")
                if not insts:
                    continue
                out = []
                for i in insts:
                    si = i.get("sync_info") or {}
                    waits = si.get("on_wait") or []
                    if len(waits) > 1:
                        for w in waits[:-1]:
                            ctr[0] += 1
                            out.append(
                                {
                                    "engine": i["engine"],
                                    "ins": [],
                                    "name": f"splitwait_{ctr[0]}",
                                    "opcode": "EventSemaphore",
                                    "outs": [],
                                    "sync_info": {
                                        "on_update": [],
                                        "on_wait": [w],
                                    },
                                }
                            )
                        si["on_wait"] = [waits[-1]]
                    out.append(i)
                b["instructions"] = out
        return json.dumps(d).encode()

    nc.to_json_bytes = patched
    return nc


def _build_graph(with_bias: bool):
    import concourse.bass as bass
    import concourse.tile as tile
    from concourse import mybir

    f32 = mybir.dt.float32
    b16 = mybir.dt.bfloat16
    f8 = mybir.dt.float8e4
    AF = mybir.ActivationFunctionType
    ALU = mybir.AluOpType
    DR = mybir.MatmulPerfMode.DoubleRow
    AX = mybir.AxisListType.X

    nc = bass.Bass()
    zt_ext = nc.declare_dram_parameter("zt", [NT, P, KC * P], f8, isOutput=False)
    zr_ext = nc.declare_dram_parameter("zr", [BL, ZD], f8, isOutput=False)
    w_ext = nc.declare_dram_parameter("w", [P, 2 * G * NW], f8, isOutput=False)
    mk_ext = nc.declare_dram_parameter("mk", [BL, 8], f32, isOutput=False)
    rl_ext = nc.declare_dram_parameter("rl", [BL, K], f32, isOutput=False)
    if with_bias:
        br_ext = nc.declare_dram_parameter("br", [1, NW], f32, isOutput=False)
    o_corr = nc.declare_dram_parameter("o_corr", [DSP, DSH], b16, isOutput=True)
    o_gram = nc.declare_dram_parameter("o_gram", [DSP, DSP], b16, isOutput=True)
    o_stat = nc.declare_dram_parameter("o_stat", [1, 1024], f32, isOutput=True)
    o_seg = nc.declare_dram_parameter("o_seg", [8, 6], f32, isOutput=True)
    o_intra = nc.declare_dram_parameter("o_intra", [P, NT], f32, isOutput=True)

    with tile.TileContext(nc) as tc:
        with (
            tc.tile_pool(name="singles", bufs=1) as singles,

            tc.tile_pool(name="stats", bufs=6) as stats,
            tc.tile_pool(name="cpool", bufs=2) as cpool,
            tc.tile_pool(name="outst", bufs=2) as outst,
            tc.tile_pool(name="pA", bufs=2, space="PSUM") as pA_pool,
            tc.tile_pool(name="pB", bufs=2, space="PSUM") as pB_pool,
            tc.tile_pool(name="pC", bufs=2, space="PSUM") as pC_pool,
            tc.tile_pool(name="pacc", bufs=1, space="PSUM") as pacc,
        ):
            # ---- persistent SBUF state ----
            W_sb = singles.tile([P, 2 * G, NW], f8)
            zT_all = singles.tile([P, NT, KC * P], f8)
            zshsp = singles.tile([P, NT, 1024], f8)   # [z_sh 768 | z_sp 256]
            sq_all = singles.tile([P, NT, DSH], f8)   # 9 * z_sh^2
            sraw_all = singles.tile([P, NT, CK], f32)  # 16x natural sims
            n2_all = singles.tile([P, NT], f32)
            mask_all = singles.tile([P, NT, 8], f32)
            mask_bf = singles.tile([P, NT, 8], b16)
            rlab_all = singles.tile([P, NT, K], f32)
            R_all = singles.tile([P, NT, 6], b16)
            strip_all = singles.tile([P, NT], f32)

            # ---- input DMAs: few big transfers (SWDGE gen is ~1us each),
            # ordered so the pieces gating the first tiles land first ----
            zr_all = singles.tile([P, NT, 2, 512], f8)
            mv_all = singles.tile([P, NT, 2], f32)

            # sync queue: first z-transposed tiles, weights group by group,
            # the remaining z tiles, then masks
            nc.sync.dma_start(
                out=zT_all[:, 0:2, :],
                in_=zt_ext[0:2].rearrange("t p c -> p t c"),
            )
            for g in range(G):
                nc.sync.dma_start(
                    out=W_sb[:, 2 * g : 2 * g + 2, :],
                    in_=w_ext[:, 2 * g * NW : (2 * g + 2) * NW].rearrange(
                        "p (j c) -> p j c", j=2
                    ),
                )
            for c0, c1 in ((2, 8), (8, 16)):
                nc.sync.dma_start(
                    out=zT_all[:, c0:c1, :],
                    in_=zt_ext[c0:c1].rearrange("t p c -> p t c"),
                )
            nc.sync.dma_start(
                out=mask_all, in_=mk_ext[:].rearrange("(t p) c -> p t c", p=P)
            )
            nc.sync.dma_start(
                out=rlab_all, in_=rl_ext[:].rearrange("(t p) k -> p t k", p=P)
            )
            # gpsimd queue: row-major z (fp8), front-loaded for stats
            for c0, c1 in ((0, 2), (2, 6), (6, 10), (10, 14), (14, 16)):
                nc.gpsimd.dma_start(
                    out=zr_all[:, c0:c1, :, :],
                    in_=zr_ext[c0 * P : c1 * P, :].rearrange(
                        "(t b) (g f) -> b t g f", b=P, g=2
                    ),
                )
            if with_bias:
                br_sb = singles.tile([1, NW], f32)
                nc.vector.dma_start(out=br_sb, in_=br_ext[:])

            # constants
            zero_t = singles.tile([P, 1], f32)
            nc.gpsimd.memset(zero_t, 0.0)
            one_t = singles.tile([P, 1], f32)
            nc.gpsimd.memset(one_t, 1.0)
            eps8_t = singles.tile([P, 1], f32)
            nc.gpsimd.memset(eps8_t, 1e-8)
            seps_t = singles.tile([P, 1], f32)
            nc.gpsimd.memset(seps_t, (S_W / S_H) ** 2 * 1e-5)
            # ones-in-column-0 stationary: PE stationary tiles are 32-wide
            # minimum, and DoubleRow only works at tile position (0, 0), so
            # each stat sum lands on out partitions 0:32 with row 0 live.
            ones32 = singles.tile([P, 2, 32], f8)
            nc.gpsimd.memset(ones32, 0.0)
            nc.scalar.copy(
                out=ones32[:, :, 0:1],
                in_=one_t[:, None, 0:1].to_broadcast([P, 2, 1]),
            )

            rstds = [None] * NT

            def emit_stats(t):
                st = stats.tile([P, 2, 6], b16, name="st")
                nc.vector.bn_stats(out=st[:, 0, :], in_=zr_all[:, t, 0, :])
                nc.vector.bn_stats(out=st[:, 1, :], in_=zr_all[:, t, 1, :])
                nc.vector.bn_aggr(out=mv_all[:, t, :], in_=st)

            def emit_rstdp(pr):
                # rstd_eff = S_H/(S_W*std) = exp(-0.5*ln(16*(var+eps)))
                # computed via Ln+Exp so the ACT engine never needs the
                # sqrt table set (keeps one act table resident all kernel)
                ts2 = slice(2 * pr, 2 * pr + 2)
                lnv = stats.tile([P, 2], f32, name="lnv")
                nc.scalar.activation(
                    out=lnv[:, :, None], in_=mv_all[:, ts2, 1:2],
                    func=AF.Ln, bias=seps_t, scale=(S_W / S_H) ** 2,
                )
                rstdp = stats.tile([P, 2], f32, name="rstdp")
                nc.scalar.activation(
                    out=rstdp, in_=lnv, func=AF.Exp, scale=-0.5, bias=zero_t
                )
                for i in range(2):
                    rstds[2 * pr + i] = rstdp[:, i : i + 1]

            def emit_mm(t):
                pA = pA_pool.tile([P, 512], f32, tag="mA", name="pA")
                pB = pB_pool.tile([P, 512], f32, tag="mB", name="pB")
                # full-bank tile: a sub-bank tile would share its bank with
                # the other buf, and start=True clears has_written bank-wide
                pC = pC_pool.tile([P, 512], f32, tag="mC", name="pC")
                for g in range(G):
                    lhsT = zT_all[:, t, 256 * g : 256 * (g + 1)].rearrange(
                        "p (j i) -> p j i", j=2
                    )
                    fl = g == 0
                    ll = g == G - 1
                    wg = W_sb[:, 2 * g : 2 * g + 2, :]
                    nc.tensor.matmul(
                        pA, lhsT, wg[:, :, 0:512],
                        start=fl, stop=ll, perf_mode=DR,
                    )
                    nc.tensor.matmul(
                        pB, lhsT, wg[:, :, 512:1024],
                        start=fl, stop=ll, perf_mode=DR,
                    )
                    nc.tensor.matmul(
                        pC[:, 0:CK], lhsT, wg[:, :, 1024:NW],
                        start=fl, stop=ll, perf_mode=DR,
                    )
                return pA, pB, pC

            def emit_copies(t, pA, pB, pC):
                rstd = rstds[t]
                # z_sh[0:512] on scalar (gpsimd cannot read PSUM)
                nc.scalar.activation(
                    out=zshsp[:, t, 0:512], in_=pA, func=AF.Copy, scale=rstd
                )
                # z_sh[512:768] + z_sp in one vector op
                nc.vector.tensor_scalar_mul(
                    zshsp[:, t, 512:1024], pB, rstd
                )
                # sims on scalar
                nc.scalar.activation(
                    out=sraw_all[:, t, :], in_=pC[:, 0:CK],
                    func=AF.Copy, scale=rstd,
                )
                if with_bias:
                    nc.vector.tensor_tensor(
                        out=zshsp[:, t, :], in0=zshsp[:, t, :],
                        in1=br_sb[0:1, 0:1024].partition_broadcast(P),
                        op=ALU.add,
                    )
                    nc.vector.tensor_tensor(
                        out=sraw_all[:, t, :], in0=sraw_all[:, t, :],
                        in1=br_sb[0:1, 1024:NW].partition_broadcast(P),
                        op=ALU.add,
                    )
                # squares: sq = 9 z_sh^2, accum gives the row norms
                nc.scalar.activation(
                    out=sq_all[:, t, :], in_=zshsp[:, t, 0:768],
                    func=AF.Square, bias=zero_t, scale=SQ_S,
                    accum_out=n2_all[:, t : t + 1],
                )

            accA = pacc.tile([P, 512], f32)
            accB = pacc.tile([P, 512], f32)

            def emit_ssq(pr):
                t2 = 2 * pr
                first = pr == 0
                last = pr == NPR - 1
                nc.tensor.matmul(
                    accA[0:32, 0:512], ones32,
                    sq_all[:, t2 : t2 + 2, 0:512],
                    start=first, stop=last,
                    perf_mode=DR, skip_group_check=True,
                )
                nc.tensor.matmul(
                    accB[0:32, 0:256], ones32,
                    sq_all[:, t2 : t2 + 2, 512:768],
                    start=first, stop=last,
                    perf_mode=DR, skip_group_check=True,
                )
                nc.tensor.matmul(
                    accB[0:32, 256:512], ones32,
                    zshsp[:, t2 : t2 + 2, 768:1024],
                    start=False, stop=last,
                    perf_mode=DR, skip_group_check=True,
                )

            def emit_chunk(ch):
                ts4 = slice(CHT * ch, CHT * (ch + 1))
                # rn = 1/(16*||z_sh||) = exp(-0.5*ln(n2*NRM_S)), sqrt-free
                lnn = cpool.tile([P, CHT], f32, name="lnn")
                nc.scalar.activation(
                    out=lnn, in_=n2_all[:, ts4], func=AF.Ln,
                    bias=eps8_t, scale=NRM_S,
                )
                rn = cpool.tile([P, CHT], f32, name="rn")
                nc.scalar.activation(
                    out=rn, in_=lnn, func=AF.Exp, scale=-0.5, bias=zero_t
                )
                sim = cpool.tile([P, CHT, CK], f32, name="simc")
                nc.gpsimd.tensor_tensor(
                    out=sim, in0=sraw_all[:, ts4, :],
                    in1=rn[:, :, None].to_broadcast([P, CHT, CK]),
                    op=ALU.mult,
                )
                t47 = cpool.tile([P, CHT, K, C], f32, name="t47")
                nc.vector.tensor_tensor(
                    out=t47,
                    in0=sim.rearrange("p t (c k) -> p t k c", k=K),
                    in1=mask_all[:, ts4, None, 0:C].to_broadcast([P, CHT, K, C]),
                    op=ALU.mult,
                )
                simK = cpool.tile([P, CHT, K], f32, name="simK")
                nc.vector.reduce_sum(out=simK, in_=t47, axis=AX)
                mx = cpool.tile([P, CHT], f32, name="mx")
                nc.vector.reduce_max(out=mx, in_=simK, axis=AX)
                dsub = cpool.tile([P, CHT, K], f32, name="dsub")
                nc.gpsimd.tensor_tensor(
                    out=dsub, in0=simK,
                    in1=mx[:, :, None].to_broadcast([P, CHT, K]),
                    op=ALU.subtract,
                )
                e = cpool.tile([P, CHT, K], f32, name="e")
                nc.scalar.activation(
                    out=e, in_=dsub, func=AF.Exp, scale=1.0 / TAU, bias=zero_t
                )
                se = cpool.tile([P, CHT], f32, name="se")
                nc.vector.reduce_sum(out=se, in_=e, axis=AX)
                rse = cpool.tile([P, CHT], f32, name="rse")
                nc.vector.reciprocal(out=rse, in_=se)
                q = cpool.tile([P, CHT, K], f32, name="q")
                nc.gpsimd.tensor_tensor(
                    out=q, in0=e,
                    in1=rse[:, :, None].to_broadcast([P, CHT, K]),
                    op=ALU.mult,
                )
                nc.scalar.copy(out=R_all[:, ts4, 0:4], in_=q)
                lg = cpool.tile([P, CHT, K], f32, name="lg")
                nc.scalar.activation(out=lg, in_=q, func=AF.Ln, bias=eps8_t)
                ql = cpool.tile([P, CHT, K], f32, name="ql")
                nc.gpsimd.tensor_tensor(out=ql, in0=q, in1=lg, op=ALU.mult)
                qls = cpool.tile([P, CHT], f32, name="qls")
                nc.vector.reduce_sum(out=qls, in_=ql, axis=AX)
                nc.gpsimd.tensor_copy(
                    out=R_all[:, ts4, 4:5], in_=qls[:, :, None]
                )
                srl = cpool.tile([P, CHT, K], f32, name="srl")
                nc.gpsimd.tensor_tensor(
                    out=srl, in0=simK, in1=rlab_all[:, ts4, :], op=ALU.add
                )
                qsr = cpool.tile([P, CHT, K], f32, name="qsr")
                nc.gpsimd.tensor_tensor(out=qsr, in0=q, in1=srl, op=ALU.mult)
                s = cpool.tile([P, CHT], f32, name="s")
                nc.vector.reduce_sum(out=s, in_=qsr, axis=AX)
                # relu(dist_w - r_w) = Relu(1 - sum q*(simK + r))
                nc.scalar.activation(
                    out=strip_all[:, ts4], in_=s, func=AF.Relu,
                    scale=-1.0, bias=one_t,
                )

            # ---- main loop (stats 2 tiles ahead) ----
            for t in range(NT):
                if t == 0:
                    for s in range(4):
                        emit_stats(s)
                        if s % 2 == 1:
                            emit_rstdp(s // 2)
                if t + 4 < NT:
                    s = t + 4
                    emit_stats(s)
                    if s % 2 == 1:
                        emit_rstdp(s // 2)
                mm = emit_mm(t)
                emit_copies(t, *mm)
                if t % 2 == 1 and t >= 3:
                    emit_ssq((t - 3) // 2)
                if t in (6, 10, 14):
                    emit_chunk((t - 6) // 4)
            emit_ssq(NPR - 1)
            emit_chunk(3)
            # seg stationaries, deferred here so the early scalar stream
            # is not blocked waiting on the (late) mask DMA
            nc.scalar.copy(out=mask_bf, in_=mask_all)
            nc.scalar.copy(
                out=R_all[:, :, 5:6],
                in_=one_t[:, None, 0:1].to_broadcast([P, NT, 1]),
            )

            # ---- corr + gram tails (reuse freed pA/pB banks) ----
            corr_done = []
            for jc in range(2):
                corrA = pA_pool.tile([P, 512], f32, tag="mA", name="corrA")
                corrBG = pB_pool.tile([P, 512], f32, tag="mB", name="corrBG")
                for pr in range(NPR):
                    t2 = 2 * pr
                    statn = zshsp[:, t2 : t2 + 2, 768 + jc * P : 768 + (jc + 1) * P]
                    nc.tensor.matmul(
                        corrA, statn, zshsp[:, t2 : t2 + 2, 0:512],
                        start=(pr == 0), stop=(pr == NPR - 1),
                        perf_mode=DR, skip_group_check=True,
                    )
                    nc.tensor.matmul(
                        corrBG[:, 0:256], statn, zshsp[:, t2 : t2 + 2, 512:768],
                        start=(pr == 0), stop=(pr == NPR - 1),
                        perf_mode=DR, skip_group_check=True,
                    )
                    nc.tensor.matmul(
                        corrBG[:, 256:512], statn, zshsp[:, t2 : t2 + 2, 768:1024],
                        start=False, stop=(pr == NPR - 1),
                        perf_mode=DR, skip_group_check=True,
                    )
                corr_sb = outst.tile([P, DSH], b16, tag="corr_sb", name="corr_sb")
                nc.scalar.copy(out=corr_sb[:, 0:512], in_=corrA)
                nc.vector.tensor_copy(out=corr_sb[:, 512:768], in_=corrBG[:, 0:256])
                gram_sb = outst.tile([P, DSP], b16, tag="gram_sb", name="gram_sb")
                nc.vector.tensor_copy(out=gram_sb, in_=corrBG[:, 256:512])
                nc.sync.dma_start(
                    out=o_corr[jc * P : (jc + 1) * P, :], in_=corr_sb
                )
                nc.gpsimd.dma_start(
                    out=o_gram[jc * P : (jc + 1) * P, :], in_=gram_sb
                )

            # ---- segment-sum matmuls (bf16) ----
            segacc = pC_pool.tile([8, 512], f32, tag="mC", name="segacc")[:, 0:6]
            for t in range(NT):
                nc.tensor.matmul(
                    segacc, mask_bf[:, t, :], R_all[:, t, :],
                    start=(t == 0), stop=(t == NT - 1),
                    skip_group_check=True,
                )
            seg_sb = outst.tile([8, 6], f32, tag="seg_sb", name="seg_sb")
            nc.scalar.copy(out=seg_sb, in_=segacc)
            nc.gpsimd.dma_start(out=o_seg[:], in_=seg_sb)

            # ---- epilogue: accumulators + strip -> DRAM ----
            stat_sb = outst.tile([1, 1024], f32, tag="stat_sb", name="stat_sb")
            nc.scalar.copy(out=stat_sb[0:1, 0:512], in_=accA[0:1, 0:512])
            nc.vector.tensor_copy(
                out=stat_sb[0:1, 512:768], in_=accB[0:1, 0:256]
            )
            nc.scalar.copy(
                out=stat_sb[0:1, 768:1024], in_=accB[0:1, 256:512]
            )
            nc.gpsimd.dma_start(out=o_stat[:], in_=stat_sb)
            nc.sync.dma_start(out=o_intra[:], in_=strip_all)

    return _split_multiwaits(nc)


def _host_prep(inputs):
    import ml_dtypes

    bf16 = ml_dtypes.bfloat16
    fp8 = ml_dtypes.float8_e4m3
    z = np.asarray(inputs["z"], dtype=np.float32)
    labels = np.asarray(inputs["labels"]).astype(np.int64)
    gamma = np.asarray(inputs["ln_gamma"], dtype=np.float32)
    beta = np.asarray(inputs["ln_beta"], dtype=np.float32)
    W_sh = np.asarray(inputs["W_sh"], dtype=np.float32)
    b_sh = np.asarray(inputs["b_sh"], dtype=np.float32)
    W_sp = np.asarray(inputs["W_sp"], dtype=np.float32)
    b_sp = np.asarray(inputs["b_sp"], dtype=np.float32)
    centers = np.asarray(inputs["centers"], dtype=np.float32)
    radii = np.asarray(inputs["ema_radii"], dtype=np.float32)

    cf = centers.reshape(CK, DSH)
    cn = cf / np.maximum(
        np.linalg.norm(cf, axis=1, keepdims=True), 1e-12
    ).astype(np.float32)
    W_all = np.concatenate([W_sh, W_sp, W_sh @ cn.T], axis=1)  # [ZD, NW]
    W_eff = (gamma[:, None] * W_all).astype(np.float32)
    # fp8 feed: [p, g, j, col] with d = (2g + j)*128 + p
    wq = np.clip(W_eff * S_W, -240, 240).astype(fp8)
    w_feed = np.ascontiguousarray(
        wq.reshape(G, 2, P, NW).transpose(2, 0, 1, 3).reshape(P, 2 * G * NW)
    )

    be = beta @ W_all + np.concatenate([b_sh, b_sp, b_sh @ cn.T])
    b_eff = (S_H * be).astype(np.float32)
    with_bias = bool(np.any(b_eff != 0.0))

    # center z rows (the projection's rank-1 mean term, folded on host)
    zc = z - z.mean(axis=1, keepdims=True)
    zq = np.clip(zc, -240, 240).astype(fp8)

    onehot = (labels[:, None] == np.arange(8)[None, :]).astype(np.float32)
    rlab = radii.reshape(C, K)[labels].astype(np.float32)  # [B, K]

    in_maps = []
    for i in range(NCORES):
        sl = slice(i * BL, (i + 1) * BL)
        # zt[t, p, kc*128 + i] = zq[t*128 + i, kc*128 + p]
        zt = (
            zq[sl]
            .reshape(NT, P, KC, P)
            .transpose(0, 3, 2, 1)
            .reshape(NT, P, KC * P)
        )
        m = {
            "zt": np.ascontiguousarray(zt),
            "zr": np.ascontiguousarray(zq[sl]),
            "w": w_feed,
            "mk": np.ascontiguousarray(onehot[sl]),
            "rl": np.ascontiguousarray(rlab[sl]),
        }
        if with_bias:
            m["br"] = np.ascontiguousarray(b_eff[None, :])
        in_maps.append(m)
    return in_maps, with_bias, cn


def _host_finish(results, cn):
    f64 = np.float64
    corr_raw = np.zeros((DSP, DSH), f64)
    gram = np.zeros((DSP, DSP), f64)
    stat = np.zeros(1024, f64)
    seg = np.zeros((8, 6), f64)
    intra_sum = 0.0
    for r in results:
        corr_raw += np.asarray(r["o_corr"]).astype(f64)
        gram += np.asarray(r["o_gram"]).astype(f64)
        stat += np.asarray(r["o_stat"])[0].astype(f64)
        seg += np.asarray(r["o_seg"]).astype(f64)
        intra_sum += float(np.asarray(r["o_intra"]).astype(f64).sum())

    ssq_sh = np.concatenate([stat[0:512], stat[512:768]]) / 9.0
    ssq_sp = np.diag(gram) / (S_H * S_H)
    sum_sp = stat[768:1024] / S_H
    corr_raw = corr_raw / (S_H * S_H)   # [DSP, DSH] = z_sp^T z_sh

    sum_q = seg[0:C, 0:4]
    qlsum_c = seg[0:C, 4]
    counts = seg[0:C, 5]

    n_sh = np.maximum(np.sqrt(ssq_sh), 1e-12)
    n_sp = np.maximum(np.sqrt(ssq_sp), 1e-12)
    corr = corr_raw.T / np.outer(n_sh, n_sp)
    L_ortho = (corr**2).mean()

    v = ssq_sp / B - (sum_sp / B) ** 2
    L_var = np.maximum(0.05 - v, 0.0).mean()

    L_intra = intra_sum / B

    p = sum_q / (sum_q.sum(-1, keepdims=True) + 1e-8)
    H_marg = -(p * np.log(p + 1e-8)).sum(-1)
    H_cond = (-qlsum_c) / np.maximum(counts, 1.0)
    valid = counts > 0
    L_bal_k = np.log(f64(K)) - H_marg + H_cond
    L_balance = np.where(valid, L_bal_k, 0.0).sum() / max(int(valid.sum()), 1)

    sim_mat = (cn @ cn.T).astype(f64)
    blkmask = 1.0 - np.kron(np.eye(C), np.ones((K, K)))
    L_overlap = (np.maximum(sim_mat - 0.3, 0.0) * blkmask).sum() / (
        blkmask.sum() + 1e-6
    )
    cnr = cn.reshape(C, K, DSH).astype(f64)
    sims_in = np.einsum("ckd,cld->ckl", cnr, cnr)
    triu = np.triu(np.ones((K, K)), 1)
    L_div = (np.maximum(sims_in - 0.8, 0.0) * triu).sum() / max(
        C * K * (K - 1) // 2, 1
    )

    L_ball = L_intra + 0.3 * L_overlap + 0.2 * L_div + 0.15 * L_balance
    loss = L_ball + 0.02 * L_ortho + 0.005 * L_var
    return np.float32(loss)


def _run_hw(nc, in_maps, trace=False, tmpdir=None):
    from concourse.bass_utils import run_bass_kernel_spmd

    res = run_bass_kernel_spmd(
        nc, in_maps, core_ids=list(range(NCORES)), trace=trace, tmpdir=tmpdir
    )
    return res


def _run_sim(nc, in_maps):
    from concourse.bass_interp import CoreSim

    outs = []
    for i, im in enumerate(in_maps):
        sim = CoreSim(nc, publish_trace=False)
        sim.assign_tensors(im)
        sim.simulate()
        outs.append(
            {k: np.array(sim.tensor(k)) for k in
             ("o_corr", "o_gram", "o_stat", "o_seg", "o_intra")}
        )
    return outs


def kernel(**inputs) -> np.ndarray:
    in_maps, with_bias, cn = _host_prep(inputs)
    if with_bias not in _GRAPH_CACHE:
        _GRAPH_CACHE[with_bias] = _build_graph(with_bias)
    nc = _GRAPH_CACHE[with_bias]
    if os.environ.get("KERNEL_BASS_SIM"):
        results = _run_sim(nc, in_maps)
    else:
        results = _run_hw(nc, in_maps).results
    return _host_finish(results, cn)


# revision 48
# speedup vs baseline: 1.8436x; 1.0472x over previous
"""Trainium2 Bass kernel for nn_AngularMultiCenterEmotionBall.

Data-parallel over batch B=16384 across 8 NeuronCores (2048 rows/core).

The projection GEMM z0 @ [W_sh | W_sp | W_sh @ c_norm.T] runs in fp8
(e4m3) with MatmulPerfMode.DoubleRow: each matmul instruction contracts
2x128 rows, doubling PE throughput vs bf16.  The host centers z
(mean-subtract, a shift the projection is equivariant to) and the
per-row 1/std LayerNorm scale is computed on-device via bn_stats and
folded into the PSUM->SBUF copies.  gamma/beta are folded into the
projection weights on the host (beta == 0 here).

Per-core device work:
  - bn_stats over centered-z rows -> rstd (the only LN stat left)
  - fp8 DoubleRow GEMM  (zc/std) @ [W_sh | W_sp | W_sh cnT] * scales
  - z_sh/z_sp stored fp8 at 16x natural scale; squares at 9x/8x via
    one ACT Square (with accum_out row-norms) and one Pool STT
  - column sums-of-squares / sums via ones-stationary DoubleRow
    matmuls packed into ONE PSUM bank at out partitions {0,32,64}
    (bank pre-cleared by a zeros-stationary matmul)
  - per-sample softmax q over the label's 4 centers, relu(dist-r)
  - segment stats (sum_q, sum q log q, counts) via one-hot matmuls
  - cross-correlation z_sp.T @ [z_sh | z_sp] in fp8 DoubleRow (the
    z_sp Gram diagonal supplies ssq_sp for the variance-floor loss)
The host sums the 8 partial outputs and finishes the scalar math
(plus the centers-only overlap/diversity losses).
"""

import os
import sys

import numpy as np

sys.path.insert(0, "/opt/trn_rl_repo")

# problem constants (hardcoded per harness contract)
B, ZD, C, K = 16384, 1024, 7, 4
DSH, DSP = 768, 256
TAU = 0.15
NCORES = 8
BL = B // NCORES          # 2048 rows per core
P = 128
NT = BL // P              # 16 row-tiles per core
CK = C * K                # 28
NW = DSH + DSP + CK       # 1052 fused output columns
KC = ZD // P              # 8 contraction chunks
G = KC // 2               # 4 DoubleRow groups (256 contraction each)
NPR = NT // 2             # 8 tile pairs
CHT = 4                   # tiles per softmax chunk
NCH = NT // CHT           # 4 chunks

S_W = 64.0                # fp8 weight scale
S_H = 16.0                # fp8 z_sh/z_sp storage scale
SQ_S = 0.1875             # ACT Square input scale: sqh = 9 * z_sh^2
NRM_S = (S_H * S_H) / (S_H * SQ_S) ** 2   # 256/9: nrm = 16*||z_sh||
CH8 = 8                   # tiles per softmax chunk

_GRAPH_CACHE = {}


def _split_multiwaits(nc):
    """Walrus codegen in this container accepts at most one semaphore wait
    per engine instruction. TileContext attaches several. Peel the extra
    waits off into standalone single-wait EventSemaphore instructions
    (what raw-bass wait_ge emits) placed just before the instruction —
    the engine is in-order, so wait(A); wait(B); op == op waiting {A,B}.
    Applied as a JSON rewrite at serialization time."""
    import json

    orig = nc.to_json_bytes

    def patched():
        d = json.loads(orig())
        ctr = [0]
        for f in d["functions"]:
            for b in f["blocks"]:
                insts = b.get("instructions")
                if not insts:
                    continue
                out = []
                for i in insts:
                    si = i.get("sync_info") or {}
                    waits = si.get("on_wait") or []
                    if len(waits) > 1:
                        for w in waits[:-1]:
                            ctr[0] += 1
                            out.append(
                                {
                                    "engine": i["engine"],
                                    "ins": [],
                                    "name": f"splitwait_{ctr[0]}",
                                    "opcode": "EventSemaphore",
                                    "outs": [],
                                    "sync_info": {
                                        "on_update": [],
                                        "on_wait": [w],
                                    },
                                }
                            )
                        si["on_wait"] = [waits[-1]]
                    out.append(i)
                b["instructions"] = out
        return json.dumps(d).encode()

    nc.to_json_bytes = patched
    return nc


def _build_graph(with_bias: bool):
    import concourse.bass as bass
    import concourse.tile as tile
    from concourse import mybir

    f32 = mybir.dt.float32
    b16 = mybir.dt.bfloat16
    f8 = mybir.dt.float8e4
    AF = mybir.ActivationFunctionType
    ALU = mybir.AluOpType
    DR = mybir.MatmulPerfMode.DoubleRow
    AX = mybir.AxisListType.X

    nc = bass.Bass()
    zt_ext = nc.declare_dram_parameter("zt", [NT, P, KC * P], f8, isOutput=False)
    zr_ext = nc.declare_dram_parameter("zr", [BL, ZD], f8, isOutput=False)
    w_ext = nc.declare_dram_parameter("w", [P, 2 * G * NW], f8, isOutput=False)
    mk_ext = nc.declare_dram_parameter("mk", [BL, 8], f32, isOutput=False)
    rl_ext = nc.declare_dram_parameter("rl", [BL, K], f32, isOutput=False)
    if with_bias:
        br_ext = nc.declare_dram_parameter("br", [1, NW], f32, isOutput=False)
    o_corr = nc.declare_dram_parameter("o_corr", [DSP, DSH], b16, isOutput=True)
    o_gram = nc.declare_dram_parameter("o_gram", [DSP, DSP], b16, isOutput=True)
    o_stat = nc.declare_dram_parameter("o_stat", [1, 1024], f32, isOutput=True)
    o_seg = nc.declare_dram_parameter("o_seg", [8, 6], f32, isOutput=True)
    o_intra = nc.declare_dram_parameter("o_intra", [P, NT], f32, isOutput=True)

    with tile.TileContext(nc) as tc:
        with (
            tc.tile_pool(name="singles", bufs=1) as singles,

            tc.tile_pool(name="stats", bufs=6) as stats,
            tc.tile_pool(name="cpool", bufs=2) as cpool,
            tc.tile_pool(name="outst", bufs=2) as outst,
            tc.tile_pool(name="pA", bufs=2, space="PSUM") as pA_pool,
            tc.tile_pool(name="pB", bufs=2, space="PSUM") as pB_pool,
            tc.tile_pool(name="pC", bufs=2, space="PSUM") as pC_pool,
            tc.tile_pool(name="pacc", bufs=1, space="PSUM") as pacc,
        ):
            # ---- persistent SBUF state ----
            W_sb = singles.tile([P, 2 * G, NW], f8)
            zT_all = singles.tile([P, NT, KC * P], f8)
            zshsp = singles.tile([P, NT, 1024], f8)   # [z_sh 768 | z_sp 256]
            sq_all = singles.tile([P, NT, DSH], f8)   # 9 * z_sh^2
            sraw_all = singles.tile([P, NT, CK], f32)  # 16x natural sims
            n2_all = singles.tile([P, NT], f32)
            mask_all = singles.tile([P, NT, 8], f32)
            mask_bf = singles.tile([P, NT, 8], b16)
            rlab_all = singles.tile([P, NT, K], f32)
            R_all = singles.tile([P, NT, 6], b16)
            strip_all = singles.tile([P, NT], f32)

            # ---- input DMAs: few big transfers (SWDGE gen is ~1us each),
            # ordered so the pieces gating the first tiles land first ----
            zr_all = singles.tile([P, NT, 2, 512], f8)
            mv_all = singles.tile([P, NT, 2], f32)

            # sync queue: first z-transposed tiles, weights group by group,
            # the remaining z tiles, then masks
            nc.sync.dma_start(
                out=zT_all[:, 0:2, :],
                in_=zt_ext[0:2].rearrange("t p c -> p t c"),
            )
            for g in range(G):
                nc.sync.dma_start(
                    out=W_sb[:, 2 * g : 2 * g + 2, :],
                    in_=w_ext[:, 2 * g * NW : (2 * g + 2) * NW].rearrange(
                        "p (j c) -> p j c", j=2
                    ),
                )
            # middle z tiles on the scalar queue so they are not stuck
            # behind the weights; the rest follows on sync
            nc.scalar.dma_start(
                out=zT_all[:, 2:8, :],
                in_=zt_ext[2:8].rearrange("t p c -> p t c"),
            )
            nc.sync.dma_start(
                out=zT_all[:, 8:16, :],
                in_=zt_ext[8:16].rearrange("t p c -> p t c"),
            )
            nc.sync.dma_start(
                out=mask_all, in_=mk_ext[:].rearrange("(t p) c -> p t c", p=P)
            )
            nc.sync.dma_start(
                out=rlab_all, in_=rl_ext[:].rearrange("(t p) k -> p t k", p=P)
            )
            # gpsimd queue: row-major z (fp8), front-loaded for stats
            for c0, c1 in ((0, 2), (2, 6), (6, 10), (10, 14), (14, 16)):
                nc.gpsimd.dma_start(
                    out=zr_all[:, c0:c1, :, :],
                    in_=zr_ext[c0 * P : c1 * P, :].rearrange(
                        "(t b) (g f) -> b t g f", b=P, g=2
                    ),
                )
            if with_bias:
                br_sb = singles.tile([1, NW], f32)
                nc.scalar.dma_start(out=br_sb, in_=br_ext[:])

            # constants
            zero_t = singles.tile([P, 1], f32)
            nc.gpsimd.memset(zero_t, 0.0)
            one_t = singles.tile([P, 1], f32)
            nc.gpsimd.memset(one_t, 1.0)
            eps8_t = singles.tile([P, 1], f32)
            nc.gpsimd.memset(eps8_t, 1e-8)
            seps_t = singles.tile([P, 1], f32)
            nc.gpsimd.memset(seps_t, (S_W / S_H) ** 2 * 1e-5)
            # ones-in-column-0 stationary: PE stationary tiles are 32-wide
            # minimum, and DoubleRow only works at tile position (0, 0), so
            # each stat sum lands on out partitions 0:32 with row 0 live.
            ones32 = singles.tile([P, 2, 32], f8)
            nc.gpsimd.memset(ones32, 0.0)
            nc.scalar.copy(
                out=ones32[:, :, 0:1],
                in_=one_t[:, None, 0:1].to_broadcast([P, 2, 1]),
            )

            rstds = [None] * NT

            def emit_stats(t):
                st = stats.tile([P, 2, 6], b16, name="st")
                nc.vector.bn_stats(out=st[:, 0, :], in_=zr_all[:, t, 0, :])
                nc.vector.bn_stats(out=st[:, 1, :], in_=zr_all[:, t, 1, :])
                nc.vector.bn_aggr(out=mv_all[:, t, :], in_=st)

            def emit_rstdp(pr):
                # rstd_eff = S_H/(S_W*std) = exp(-0.5*ln(16*(var+eps)))
                # computed via Ln+Exp so the ACT engine never needs the
                # sqrt table set (keeps one act table resident all kernel)
                ts2 = slice(2 * pr, 2 * pr + 2)
                lnv = stats.tile([P, 2], f32, name="lnv")
                nc.scalar.activation(
                    out=lnv[:, :, None], in_=mv_all[:, ts2, 1:2],
                    func=AF.Ln, bias=seps_t, scale=(S_W / S_H) ** 2,
                )
                rstdp = stats.tile([P, 2], f32, name="rstdp")
                nc.scalar.activation(
                    out=rstdp, in_=lnv, func=AF.Exp, scale=-0.5, bias=zero_t
                )
                for i in range(2):
                    rstds[2 * pr + i] = rstdp[:, i : i + 1]

            def emit_mm(t):
                pA = pA_pool.tile([P, 512], f32, tag="mA", name="pA")
                pB = pB_pool.tile([P, 512], f32, tag="mB", name="pB")
                # full-bank tile: a sub-bank tile would share its bank with
                # the other buf, and start=True clears has_written bank-wide
                pC = pC_pool.tile([P, 512], f32, tag="mC", name="pC")
                for g in range(G):
                    lhsT = zT_all[:, t, 256 * g : 256 * (g + 1)].rearrange(
                        "p (j i) -> p j i", j=2
                    )
                    fl = g == 0
                    ll = g == G - 1
                    wg = W_sb[:, 2 * g : 2 * g + 2, :]
                    nc.tensor.matmul(
                        pA, lhsT, wg[:, :, 0:512],
                        start=fl, stop=ll, perf_mode=DR,
                    )
                    nc.tensor.matmul(
                        pB, lhsT, wg[:, :, 512:1024],
                        start=fl, stop=ll, perf_mode=DR,
                    )
                    nc.tensor.matmul(
                        pC[:, 0:CK], lhsT, wg[:, :, 1024:NW],
                        start=fl, stop=ll, perf_mode=DR,
                    )
                return pA, pB, pC

            def emit_copies(t, pA, pB, pC):
                rstd = rstds[t]
                # z_sh[0:512] on scalar (gpsimd cannot read PSUM)
                nc.scalar.activation(
                    out=zshsp[:, t, 0:512], in_=pA, func=AF.Copy, scale=rstd
                )
                # z_sh[512:768] + z_sp in one vector op
                nc.vector.tensor_scalar_mul(
                    zshsp[:, t, 512:1024], pB, rstd
                )
                # sims on scalar
                nc.scalar.activation(
                    out=sraw_all[:, t, :], in_=pC[:, 0:CK],
                    func=AF.Copy, scale=rstd,
                )
                if with_bias:
                    nc.vector.tensor_tensor(
                        out=zshsp[:, t, :], in0=zshsp[:, t, :],
                        in1=br_sb[0:1, 0:1024].partition_broadcast(P),
                        op=ALU.add,
                    )
                    nc.vector.tensor_tensor(
                        out=sraw_all[:, t, :], in0=sraw_all[:, t, :],
                        in1=br_sb[0:1, 1024:NW].partition_broadcast(P),
                        op=ALU.add,
                    )
                # squares: sq = 9 z_sh^2, accum gives the row norms
                nc.scalar.activation(
                    out=sq_all[:, t, :], in_=zshsp[:, t, 0:768],
                    func=AF.Square, bias=zero_t, scale=SQ_S,
                    accum_out=n2_all[:, t : t + 1],
                )

            accA = pacc.tile([P, 512], f32)
            accB = pacc.tile([P, 512], f32)

            def emit_ssq(pr):
                t2 = 2 * pr
                first = pr == 0
                last = pr == NPR - 1
                nc.tensor.matmul(
                    accA[0:32, 0:512], ones32,
                    sq_all[:, t2 : t2 + 2, 0:512],
                    start=first, stop=last,
                    perf_mode=DR, skip_group_check=True,
                )
                nc.tensor.matmul(
                    accB[0:32, 0:256], ones32,
                    sq_all[:, t2 : t2 + 2, 512:768],
                    start=first, stop=last,
                    perf_mode=DR, skip_group_check=True,
                )
                nc.tensor.matmul(
                    accB[0:32, 256:512], ones32,
                    zshsp[:, t2 : t2 + 2, 768:1024],
                    start=False, stop=last,
                    perf_mode=DR, skip_group_check=True,
                )

            def emit_chunk(ch):
                ts8 = slice(CH8 * ch, CH8 * (ch + 1))
                # rn = 1/(16*||z_sh||) = exp(-0.5*ln(n2*NRM_S)), sqrt-free
                lnn = cpool.tile([P, CH8], f32, name="lnn")
                nc.scalar.activation(
                    out=lnn, in_=n2_all[:, ts8], func=AF.Ln,
                    bias=eps8_t, scale=NRM_S,
                )
                rn = cpool.tile([P, CH8], f32, name="rn")
                nc.scalar.activation(
                    out=rn, in_=lnn, func=AF.Exp, scale=-0.5, bias=zero_t
                )
                sim = cpool.tile([P, CH8, CK], f32, name="simc")
                nc.gpsimd.tensor_tensor(
                    out=sim, in0=sraw_all[:, ts8, :],
                    in1=rn[:, :, None].to_broadcast([P, CH8, CK]),
                    op=ALU.mult,
                )
                t47 = cpool.tile([P, CH8, K, C], f32, name="t47")
                nc.vector.tensor_tensor(
                    out=t47,
                    in0=sim.rearrange("p t (c k) -> p t k c", k=K),
                    in1=mask_all[:, ts8, None, 0:C].to_broadcast([P, CH8, K, C]),
                    op=ALU.mult,
                )
                simK = cpool.tile([P, CH8, K], f32, name="simK")
                nc.vector.reduce_sum(out=simK, in_=t47, axis=AX)
                # softmax without max-subtraction: |simK/TAU| <= ~1.3
                e = cpool.tile([P, CH8, K], f32, name="e")
                nc.scalar.activation(
                    out=e, in_=simK, func=AF.Exp, scale=1.0 / TAU, bias=zero_t
                )
                se = cpool.tile([P, CH8], f32, name="se")
                nc.vector.reduce_sum(out=se, in_=e, axis=AX)
                rse = cpool.tile([P, CH8], f32, name="rse")
                nc.vector.reciprocal(out=rse, in_=se)
                q = cpool.tile([P, CH8, K], f32, name="q")
                nc.gpsimd.tensor_tensor(
                    out=q, in0=e,
                    in1=rse[:, :, None].to_broadcast([P, CH8, K]),
                    op=ALU.mult,
                )
                nc.scalar.copy(out=R_all[:, ts8, 0:4], in_=q)
                qs = cpool.tile([P, CH8, K], f32, name="qs")
                nc.gpsimd.tensor_tensor(out=qs, in0=q, in1=simK, op=ALU.mult)
                ds = cpool.tile([P, CH8], f32, name="ds")
                nc.vector.reduce_sum(out=ds, in_=qs, axis=AX)
                qr = cpool.tile([P, CH8, K], f32, name="qr")
                nc.gpsimd.tensor_tensor(
                    out=qr, in0=q, in1=rlab_all[:, ts8, :], op=ALU.mult
                )
                rw = cpool.tile([P, CH8], f32, name="rw")
                nc.vector.reduce_sum(out=rw, in_=qr, axis=AX)
                # sum q*ln q = ds/TAU - ln(se)  (exact softmax identity)
                lnse = cpool.tile([P, CH8], f32, name="lnse")
                nc.scalar.activation(
                    out=lnse, in_=se, func=AF.Ln, bias=eps8_t
                )
                qls = cpool.tile([P, CH8], f32, name="qls")
                nc.vector.scalar_tensor_tensor(
                    out=qls, in0=ds, scalar=1.0 / TAU, in1=lnse,
                    op0=ALU.mult, op1=ALU.subtract,
                )
                nc.gpsimd.tensor_copy(
                    out=R_all[:, ts8, 4:5], in_=qls[:, :, None]
                )
                s = cpool.tile([P, CH8], f32, name="s")
                nc.gpsimd.tensor_tensor(out=s, in0=ds, in1=rw, op=ALU.add)
                # relu(dist_w - r_w) = Relu(1 - ds - rw)
                nc.scalar.activation(
                    out=strip_all[:, ts8], in_=s, func=AF.Relu,
                    scale=-1.0, bias=one_t,
                )

            # ---- main loop (stats 2 tiles ahead) ----
            for t in range(NT):
                if t == 0:
                    for s in range(4):
                        emit_stats(s)
                        if s % 2 == 1:
                            emit_rstdp(s // 2)
                if t + 4 < NT:
                    s = t + 4
                    emit_stats(s)
                    if s % 2 == 1:
                        emit_rstdp(s // 2)
                mm = emit_mm(t)
                emit_copies(t, *mm)
                if t % 2 == 1 and t >= 3:
                    emit_ssq((t - 3) // 2)
                if t == 10:
                    emit_chunk(0)
            emit_ssq(NPR - 1)
            emit_chunk(1)
            # seg stationaries, deferred here so the early scalar stream
            # is not blocked waiting on the (late) mask DMA
            nc.scalar.copy(out=mask_bf, in_=mask_all)
            nc.scalar.copy(
                out=R_all[:, :, 5:6],
                in_=one_t[:, None, 0:1].to_broadcast([P, NT, 1]),
            )

            # ---- corr + gram tails (reuse freed pA/pB banks) ----
            corr_done = []
            for jc in range(2):
                corrA = pA_pool.tile([P, 512], f32, tag="mA", name="corrA")
                corrBG = pB_pool.tile([P, 512], f32, tag="mB", name="corrBG")
                for pr in range(NPR):
                    t2 = 2 * pr
                    statn = zshsp[:, t2 : t2 + 2, 768 + jc * P : 768 + (jc + 1) * P]
                    nc.tensor.matmul(
                        corrA, statn, zshsp[:, t2 : t2 + 2, 0:512],
                        start=(pr == 0), stop=(pr == NPR - 1),
                        perf_mode=DR, skip_group_check=True,
                    )
                    nc.tensor.matmul(
                        corrBG[:, 0:256], statn, zshsp[:, t2 : t2 + 2, 512:768],
                        start=(pr == 0), stop=(pr == NPR - 1),
                        perf_mode=DR, skip_group_check=True,
                    )
                    nc.tensor.matmul(
                        corrBG[:, 256:512], statn, zshsp[:, t2 : t2 + 2, 768:1024],
                        start=False, stop=(pr == NPR - 1),
                        perf_mode=DR, skip_group_check=True,
                    )
                corr_sb = outst.tile([P, DSH], b16, tag="corr_sb", name="corr_sb")
                nc.scalar.copy(out=corr_sb[:, 0:512], in_=corrA)
                nc.vector.tensor_copy(out=corr_sb[:, 512:768], in_=corrBG[:, 0:256])
                gram_sb = outst.tile([P, DSP], b16, tag="gram_sb", name="gram_sb")
                nc.vector.tensor_copy(out=gram_sb, in_=corrBG[:, 256:512])
                nc.sync.dma_start(
                    out=o_corr[jc * P : (jc + 1) * P, :], in_=corr_sb
                )
                nc.gpsimd.dma_start(
                    out=o_gram[jc * P : (jc + 1) * P, :], in_=gram_sb
                )

            # ---- segment-sum matmuls (bf16) ----
            segacc = pC_pool.tile([8, 512], f32, tag="mC", name="segacc")[:, 0:6]
            for t in range(NT):
                nc.tensor.matmul(
                    segacc, mask_bf[:, t, :], R_all[:, t, :],
                    start=(t == 0), stop=(t == NT - 1),
                    skip_group_check=True,
                )
            seg_sb = outst.tile([8, 6], f32, tag="seg_sb", name="seg_sb")
            nc.scalar.copy(out=seg_sb, in_=segacc)
            nc.gpsimd.dma_start(out=o_seg[:], in_=seg_sb)

            # ---- epilogue: accumulators + strip -> DRAM ----
            stat_sb = outst.tile([1, 1024], f32, tag="stat_sb", name="stat_sb")
            nc.scalar.copy(out=stat_sb[0:1, 0:512], in_=accA[0:1, 0:512])
            nc.vector.tensor_copy(
                out=stat_sb[0:1, 512:768], in_=accB[0:1, 0:256]
            )
            nc.scalar.copy(
                out=stat_sb[0:1, 768:1024], in_=accB[0:1, 256:512]
            )
            nc.gpsimd.dma_start(out=o_stat[:], in_=stat_sb)
            nc.sync.dma_start(out=o_intra[:], in_=strip_all)

    return _split_multiwaits(nc)


def _host_prep(inputs):
    import ml_dtypes

    bf16 = ml_dtypes.bfloat16
    fp8 = ml_dtypes.float8_e4m3
    z = np.asarray(inputs["z"], dtype=np.float32)
    labels = np.asarray(inputs["labels"]).astype(np.int64)
    gamma = np.asarray(inputs["ln_gamma"], dtype=np.float32)
    beta = np.asarray(inputs["ln_beta"], dtype=np.float32)
    W_sh = np.asarray(inputs["W_sh"], dtype=np.float32)
    b_sh = np.asarray(inputs["b_sh"], dtype=np.float32)
    W_sp = np.asarray(inputs["W_sp"], dtype=np.float32)
    b_sp = np.asarray(inputs["b_sp"], dtype=np.float32)
    centers = np.asarray(inputs["centers"], dtype=np.float32)
    radii = np.asarray(inputs["ema_radii"], dtype=np.float32)

    cf = centers.reshape(CK, DSH)
    cn = cf / np.maximum(
        np.linalg.norm(cf, axis=1, keepdims=True), 1e-12
    ).astype(np.float32)
    W_all = np.concatenate([W_sh, W_sp, W_sh @ cn.T], axis=1)  # [ZD, NW]
    W_eff = (gamma[:, None] * W_all).astype(np.float32)
    # fp8 feed: [p, g, j, col] with d = (2g + j)*128 + p
    wq = np.clip(W_eff * S_W, -240, 240).astype(fp8)
    w_feed = np.ascontiguousarray(
        wq.reshape(G, 2, P, NW).transpose(2, 0, 1, 3).reshape(P, 2 * G * NW)
    )

    be = beta @ W_all + np.concatenate([b_sh, b_sp, b_sh @ cn.T])
    b_eff = (S_H * be).astype(np.float32)
    with_bias = bool(np.any(b_eff != 0.0))

    # center z rows (the projection's rank-1 mean term, folded on host)
    zc = z - z.mean(axis=1, keepdims=True)
    zq = np.clip(zc, -240, 240).astype(fp8)

    onehot = (labels[:, None] == np.arange(8)[None, :]).astype(np.float32)
    rlab = radii.reshape(C, K)[labels].astype(np.float32)  # [B, K]

    in_maps = []
    for i in range(NCORES):
        sl = slice(i * BL, (i + 1) * BL)
        # zt[t, p, kc*128 + i] = zq[t*128 + i, kc*128 + p]
        zt = (
            zq[sl]
            .reshape(NT, P, KC, P)
            .transpose(0, 3, 2, 1)
            .reshape(NT, P, KC * P)
        )
        m = {
            "zt": np.ascontiguousarray(zt),
            "zr": np.ascontiguousarray(zq[sl]),
            "w": w_feed,
            "mk": np.ascontiguousarray(onehot[sl]),
            "rl": np.ascontiguousarray(rlab[sl]),
        }
        if with_bias:
            m["br"] = np.ascontiguousarray(b_eff[None, :])
        in_maps.append(m)
    return in_maps, with_bias, cn


def _host_finish(results, cn):
    f64 = np.float64
    corr_raw = np.zeros((DSP, DSH), f64)
    gram = np.zeros((DSP, DSP), f64)
    stat = np.zeros(1024, f64)
    seg = np.zeros((8, 6), f64)
    intra_sum = 0.0
    for r in results:
        corr_raw += np.asarray(r["o_corr"]).astype(f64)
        gram += np.asarray(r["o_gram"]).astype(f64)
        stat += np.asarray(r["o_stat"])[0].astype(f64)
        seg += np.asarray(r["o_seg"]).astype(f64)
        intra_sum += float(np.asarray(r["o_intra"]).astype(f64).sum())

    ssq_sh = np.concatenate([stat[0:512], stat[512:768]]) / 9.0
    ssq_sp = np.diag(gram) / (S_H * S_H)
    sum_sp = stat[768:1024] / S_H
    corr_raw = corr_raw / (S_H * S_H)   # [DSP, DSH] = z_sp^T z_sh

    sum_q = seg[0:C, 0:4]
    qlsum_c = seg[0:C, 4]
    counts = seg[0:C, 5]

    n_sh = np.maximum(np.sqrt(ssq_sh), 1e-12)
    n_sp = np.maximum(np.sqrt(ssq_sp), 1e-12)
    corr = corr_raw.T / np.outer(n_sh, n_sp)
    L_ortho = (corr**2).mean()

    v = ssq_sp / B - (sum_sp / B) ** 2
    L_var = np.maximum(0.05 - v, 0.0).mean()

    L_intra = intra_sum / B

    p = sum_q / (sum_q.sum(-1, keepdims=True) + 1e-8)
    H_marg = -(p * np.log(p + 1e-8)).sum(-1)
    H_cond = (-qlsum_c) / np.maximum(counts, 1.0)
    valid = counts > 0
    L_bal_k = np.log(f64(K)) - H_marg + H_cond
    L_balance = np.where(valid, L_bal_k, 0.0).sum() / max(int(valid.sum()), 1)

    sim_mat = (cn @ cn.T).astype(f64)
    blkmask = 1.0 - np.kron(np.eye(C), np.ones((K, K)))
    L_overlap = (np.maximum(sim_mat - 0.3, 0.0) * blkmask).sum() / (
        blkmask.sum() + 1e-6
    )
    cnr = cn.reshape(C, K, DSH).astype(f64)
    sims_in = np.einsum("ckd,cld->ckl", cnr, cnr)
    triu = np.triu(np.ones((K, K)), 1)
    L_div = (np.maximum(sims_in - 0.8, 0.0) * triu).sum() / max(
        C * K * (K - 1) // 2, 1
    )

    L_ball = L_intra + 0.3 * L_overlap + 0.2 * L_div + 0.15 * L_balance
    loss = L_ball + 0.02 * L_ortho + 0.005 * L_var
    return np.float32(loss)


def _run_hw(nc, in_maps, trace=False, tmpdir=None):
    from concourse.bass_utils import run_bass_kernel_spmd

    res = run_bass_kernel_spmd(
        nc, in_maps, core_ids=list(range(NCORES)), trace=trace, tmpdir=tmpdir
    )
    return res


def _run_sim(nc, in_maps):
    from concourse.bass_interp import CoreSim

    outs = []
    for i, im in enumerate(in_maps):
        sim = CoreSim(nc, publish_trace=False)
        sim.assign_tensors(im)
        sim.simulate()
        outs.append(
            {k: np.array(sim.tensor(k)) for k in
             ("o_corr", "o_gram", "o_stat", "o_seg", "o_intra")}
        )
    return outs


def kernel(**inputs) -> np.ndarray:
    in_maps, with_bias, cn = _host_prep(inputs)
    if with_bias not in _GRAPH_CACHE:
        _GRAPH_CACHE[with_bias] = _build_graph(with_bias)
    nc = _GRAPH_CACHE[with_bias]
    if os.environ.get("KERNEL_BASS_SIM"):
        results = _run_sim(nc, in_maps)
    else:
        results = _run_hw(nc, in_maps).results
    return _host_finish(results, cn)
